# revision 15
# baseline (speedup 1.0000x reference)
"""DeepAR Trainium2 Bass kernel.

Strategy (hardcoded from spec nn_DeepAR_90374701843258):
  B=32, LIN=96, LOUT=24, N=256, E=32, H=64, T-1=119 steps, 8 cores.
  Data-parallel over B: 4 batch rows per core -> per-core batch BN=1024.
  Layout: "folded" [128, 512] tiles everywhere: partition p<64 = H-unit p of
  batch half 0 (cols 0:512 of the 1024 batch), p>=64 = H-unit p-64 of half 1.

  Algebra:
   - embedding + layer0 input proj collapse to rank-1: pre0 = x*w_eff + b_eff
     (w_eff = Wih0 @ embed_W), injected as extra contraction rows in the
     recurrent matmul (x and ones rows live in the xo tile).
   - sigmoid(z) = (tanh(z/2)+1)/2: i,f,o gate weights are pre-scaled by 0.5 so
     ALL four gates use one Tanh activation pass. h is stored as Hs=2h (the
     next-layer weights absorb the 0.5), cell state as s=2c.
     s' = 0.5*(tf+1)*s + (ti+1)*tg ; Hs = (to+1)*tanh(0.5*s')
     -> 4 fused scalar_tensor_tensor DVE ops per layer.
"""

import numpy as np

B, LIN, LOUT, N, E, H = 32, 96, 24, 256, 32, 64
T = LIN + LOUT
TS = T - 1            # 119
NCORES = 8
BL = B // NCORES      # 4
BN = BL * N           # 1024
HALF = 512
NCHUNK = BN // 128    # 8

_cache = {}


def _xo_init():
    xoi = np.zeros((128, HALF), np.float32)
    xoi[1] = 1.0
    xoi[32] = 1.0
    xoi[65] = 1.0
    return xoi


def _pack_weights(inp):
    """Host-side weight prep (tiny arrays). Returns dict of np arrays."""
    import ml_dtypes
    bf16 = ml_dtypes.bfloat16
    f32 = np.float32

    Wih0, Whh0 = inp["Wih0"].astype(f32), inp["Whh0"].astype(f32)
    Wih1, Whh1 = inp["Wih1"].astype(f32), inp["Whh1"].astype(f32)
    w_eff = (Wih0 @ inp["embed_W"].astype(f32))[:, 0]
    b_eff = Wih0 @ inp["embed_b"].astype(f32) + inp["bih0"] + inp["bhh0"]
    b1 = (inp["bih1"] + inp["bhh1"]).astype(f32)

    sc = np.ones(4 * H, f32)
    sc[: 2 * H] = 0.5      # i, f
    sc[3 * H:] = 0.5       # o
    habs = 0.5             # h stored as 2h

    def two_copies(top):
        w = np.zeros((128, 4 * H), f32)
        w[0:64] = top
        w[64:128] = top
        return w

    WH0 = two_copies(Whh0.T * (habs * sc)[None, :])
    WI1 = two_copies(Wih1.T * (habs * sc)[None, :])
    WH1 = two_copies(Whh1.T * (habs * sc)[None, :])

    WX0 = np.zeros((128, 4 * H), f32)
    WX0[64] = w_eff * sc
    WX0[65] = b_eff * sc
    WX0[0] = w_eff * sc
    WX0[1] = b_eff * sc

    B1T = np.zeros((128, 4 * H), f32)
    B1T[32] = b1 * sc

    HD = np.zeros((128, 2), f32)
    HD[0:64, 0] = inp["mu_W"].astype(f32)[0] * habs
    HD[0:64, 1] = inp["sigma_W"].astype(f32)[0] * habs
    HD[64:128] = HD[0:64]

    ID = np.eye(128, dtype=f32)

    return {
        "WH0": WH0.astype(bf16), "WX0": WX0.astype(bf16),
        "WI1": WI1.astype(bf16), "WH1": WH1.astype(bf16),
        "B1T": B1T.astype(bf16), "HD": HD.astype(bf16),
        "ID": ID.astype(bf16),
        "XOINIT": _xo_init().astype(bf16),
        "mu_b": float(inp["mu_b"][0]), "sigma_b": float(inp["sigma_b"][0]),
    }


def _build(mu_b, sigma_b):
    """Build the per-core bass program (SPMD: identical on all cores)."""
    from contextlib import ExitStack
    import concourse.bass as bass
    import concourse.mybir as mybir
    import concourse.tile as tile
    from concourse import bacc

    dt = mybir.dt
    AF = mybir.ActivationFunctionType
    OP = mybir.AluOpType

    nc = bacc.Bacc()

    # ---- I/O declarations -------------------------------------------------
    hist = nc.declare_dram_parameter("hist", [BL, LIN, N], dt.float32, isOutput=False)
    fut = nc.declare_dram_parameter("fut", [BL, LOUT, N], dt.float32, isOutput=False)
    hmask = nc.declare_dram_parameter("hmask", [BL, LIN, N], dt.float32, isOutput=False)
    fmask = nc.declare_dram_parameter("fmask", [BL, LOUT, N], dt.float32, isOutput=False)
    epsin = nc.declare_dram_parameter("epsin", [BL, TS, N], dt.float32, isOutput=False)
    wWH0 = nc.declare_dram_parameter("WH0", [128, 4 * H], dt.bfloat16, isOutput=False)
    wWX0 = nc.declare_dram_parameter("WX0", [128, 4 * H], dt.bfloat16, isOutput=False)
    wWI1 = nc.declare_dram_parameter("WI1", [128, 4 * H], dt.bfloat16, isOutput=False)
    wWH1 = nc.declare_dram_parameter("WH1", [128, 4 * H], dt.bfloat16, isOutput=False)
    wB1T = nc.declare_dram_parameter("B1T", [128, 4 * H], dt.bfloat16, isOutput=False)
    wHD = nc.declare_dram_parameter("HD", [128, 2], dt.bfloat16, isOutput=False)
    wID = nc.declare_dram_parameter("ID", [128, 128], dt.bfloat16, isOutput=False)
    wXOI = nc.declare_dram_parameter("XOINIT", [128, HALF], dt.bfloat16, isOutput=False)

    o_preds = nc.declare_dram_parameter("preds", [BL, TS, N], dt.float32, isOutput=True)
    o_reals = nc.declare_dram_parameter("reals", [BL, TS, N], dt.float32, isOutput=True)
    o_mus = nc.declare_dram_parameter("musv", [BL, TS, N], dt.float32, isOutput=True)
    o_sigs = nc.declare_dram_parameter("sigmasv", [BL, TS, N], dt.float32, isOutput=True)
    o_mask = nc.declare_dram_parameter("maskv", [BL, TS, N], dt.float32, isOutput=True)

    musig_d = nc.dram_tensor("musig", [128, 2, BN], dt.bfloat16)

    with ExitStack() as ctx:
        tc = ctx.enter_context(tile.TileContext(nc))
        persist = ctx.enter_context(tc.tile_pool(name="persist", bufs=1))
        work = ctx.enter_context(tc.tile_pool(name="work", bufs=3))
        psl0 = ctx.enter_context(tc.tile_pool(name="psl0", bufs=1, space="PSUM"))
        psl1 = ctx.enter_context(tc.tile_pool(name="psl1", bufs=1, space="PSUM"))

        # ---- constants / weights into SBUF -------------------------------
        WH0 = persist.tile([128, 4 * H], dt.bfloat16, tag="WH0")
        WX0 = persist.tile([128, 4 * H], dt.bfloat16, tag="WX0")
        WI1 = persist.tile([128, 4 * H], dt.bfloat16, tag="WI1")
        WH1 = persist.tile([128, 4 * H], dt.bfloat16, tag="WH1")
        B1T = persist.tile([128, 4 * H], dt.bfloat16, tag="B1T")
        HD = persist.tile([128, 2], dt.bfloat16, tag="HD")
        ID = persist.tile([128, 128], dt.bfloat16, tag="ID")
        for t_, d_ in [(WH0, wWH0), (WX0, wWX0), (WI1, wWI1), (WH1, wWH1),
                       (B1T, wB1T), (HD, wHD), (ID, wID)]:
            nc.sync.dma_start(out=t_[:], in_=d_[:])

        c_half = persist.tile([128, 1], dt.float32, tag="c_half")
        nc.vector.memset(c_half, 0.5)
        c_neg1 = persist.tile([128, 1], dt.float32, tag="c_neg1")
        nc.vector.memset(c_neg1, -1.0)
        c_sigb = persist.tile([128, 1], dt.float32, tag="c_sigb")
        nc.vector.memset(c_sigb, sigma_b)

        # ---- persistent state tiles ---------------------------------------
        ht0 = persist.tile([128, HALF], dt.bfloat16, tag="ht0")   # Hs0 = 2*h0
        ht1 = persist.tile([128, HALF], dt.bfloat16, tag="ht1")   # Hs1 = 2*h1
        s0t = persist.tile([128, HALF], dt.bfloat16, tag="s0t")   # s0 = 2*c0
        s1t = persist.tile([128, HALF], dt.bfloat16, tag="s1t")   # s1 = 2*c1
        for t_ in (ht0, ht1, s0t, s1t):
            nc.vector.memset(t_, 0.0)

        tg0 = persist.tile([128, 4 * HALF], dt.bfloat16, tag="tg0")
        tg1 = persist.tile([128, 4 * HALF], dt.bfloat16, tag="tg1")
        a0 = persist.tile([128, HALF], dt.bfloat16, tag="a0")
        b0 = persist.tile([128, HALF], dt.bfloat16, tag="b0")
        a1 = persist.tile([128, HALF], dt.bfloat16, tag="a1")
        b1t_ = persist.tile([128, HALF], dt.bfloat16, tag="b1t_")
        tc0 = persist.tile([128, HALF], dt.bfloat16, tag="tc0")
        tc1 = persist.tile([128, HALF], dt.bfloat16, tag="tc1")

        xo = [persist.tile([128, HALF], dt.bfloat16, tag=f"xo{i}", name=f"xo{i}")
              for i in range(3)]
        for xot in xo:
            nc.sync.dma_start(out=xot[:], in_=wXOI[:])

        musig_sb = [persist.tile([2, BN], dt.bfloat16, tag=f"ms{i}", name=f"ms{i}")
                    for i in range(2)]

        xt = persist.tile([TS, BN], dt.bfloat16, tag="xt")

        # stats + raw data per chunk (kept for the post-pass)
        full_c, mv_c, stdev_c, istd_c = [], [], [], []

        # ---- pre-pass: stats, normalize, transpose x ----------------------
        for c in range(NCHUNK):
            b_, n0 = c // 2, (c % 2) * 128
            fc = persist.tile([128, T], dt.float32, tag=f"full{c}")
            nc.sync.dma_start(out=fc[:, 0:LIN],
                              in_=hist[b_, :, n0:n0 + 128].rearrange("t n -> n t"))
            nc.sync.dma_start(out=fc[:, LIN:T],
                              in_=fut[b_, :, n0:n0 + 128].rearrange("t n -> n t"))
            st6 = work.tile([128, 6], dt.float32, tag="st6")
            mv = persist.tile([128, 2], dt.float32, tag=f"mv{c}")
            nc.vector.bn_stats(out=st6, in_=fc[:, 0:LIN])
            nc.vector.bn_aggr(out=mv, in_=st6)
            # stdev = sqrt(var+1e-5) with one Newton refinement (Sqrt table is
            # low precision); istd = 1/stdev via DVE reciprocal.
            veps = work.tile([128, 1], dt.float32, tag="veps")
            nc.vector.tensor_scalar(out=veps, in0=mv[:, 1:2], scalar1=1e-5,
                                    scalar2=None, op0=OP.add)
            y0 = work.tile([128, 1], dt.float32, tag="y0")
            nc.scalar.activation(y0, veps, AF.Sqrt)
            r0 = work.tile([128, 1], dt.float32, tag="r0")
            nc.vector.reciprocal(r0, y0)
            yy = work.tile([128, 1], dt.float32, tag="yy")
            nc.vector.tensor_tensor(out=yy, in0=y0, in1=y0, op=OP.mult)
            e_ = work.tile([128, 1], dt.float32, tag="e_")
            nc.vector.tensor_tensor(out=e_, in0=veps, in1=yy, op=OP.subtract)
            d_ = work.tile([128, 1], dt.float32, tag="d_")
            nc.vector.scalar_tensor_tensor(out=d_, in0=e_, scalar=0.5, in1=r0,
                                           op0=OP.mult, op1=OP.mult)
            sd = persist.tile([128, 1], dt.float32, tag=f"sd{c}")
            nc.vector.tensor_tensor(out=sd, in0=y0, in1=d_, op=OP.add)
            isd = persist.tile([128, 1], dt.float32, tag=f"isd{c}")
            nc.vector.reciprocal(isd, sd)
            full_c.append(fc); mv_c.append(mv); stdev_c.append(sd); istd_c.append(isd)

            # normalized x for steps 0..118, bf16
            xn = work.tile([128, TS], dt.bfloat16, tag="xn")
            nc.vector.tensor_scalar(out=xn, in0=fc[:, 0:TS], scalar1=mv[:, 0:1],
                                    scalar2=isd, op0=OP.subtract, op1=OP.mult)
            # transpose to [TS, 128] and write into xt with halves swapped
            pt = psl0.tile([TS, 128], dt.bfloat16, tag="l0", name="pt")
            nc.tensor.transpose(pt, xn, ID)
            xtcol = (1 - c // 4) * HALF + (c % 4) * 128
            nc.vector.tensor_copy(xt[:, xtcol:xtcol + 128], pt)

        # ---- main loop ----------------------------------------------------
        GS = [slice(X * 64, (X + 1) * 64) for X in range(4)]  # gate cols in W packs

        def lstm_matmuls(ps, Wh, hsrc, layer1):
            for X in range(4):
                psX = ps[:, X * HALF:(X + 1) * HALF]
                g = GS[X]
                # half 0
                nc.tensor.matmul(psX[0:64, :], lhsT=Wh[0:64, g], rhs=hsrc[0:64, :],
                                 start=True, stop=False, tile_position=(0, 0))
                if layer1:
                    nc.tensor.matmul(psX[0:64, :], lhsT=WH1[0:64, g],
                                     rhs=ht1[0:64, :], start=False, stop=False,
                                     tile_position=(0, 0))
                    nc.tensor.matmul(psX[0:64, :], lhsT=B1T[32:33, g],
                                     rhs=xo_t[32:33, :], start=False, stop=True,
                                     tile_position=(32, 0))
                else:
                    nc.tensor.matmul(psX[0:64, :], lhsT=WX0[64:66, g],
                                     rhs=xo_t[64:66, :], start=False, stop=True,
                                     tile_position=(64, 0))
                # half 1
                nc.tensor.matmul(psX[64:128, :], lhsT=Wh[64:128, g],
                                 rhs=hsrc[64:128, :], start=True, stop=False,
                                 tile_position=(64, 64))
                if layer1:
                    nc.tensor.matmul(psX[64:128, :], lhsT=WH1[64:128, g],
                                     rhs=ht1[64:128, :], start=False, stop=False,
                                     tile_position=(64, 64))
                    nc.tensor.matmul(psX[64:128, :], lhsT=B1T[32:33, g],
                                     rhs=xo_t[32:33, :], start=False, stop=True,
                                     tile_position=(32, 64))
                else:
                    nc.tensor.matmul(psX[64:128, :], lhsT=WX0[0:2, g],
                                     rhs=xo_t[0:2, :], start=False, stop=True,
                                     tile_position=(0, 64))

        def cell_update(tgb, st, ab, bb, tcb, htile):
            ti = tgb[:, 0:HALF]
            tf = tgb[:, HALF:2 * HALF]
            tg_ = tgb[:, 2 * HALF:3 * HALF]
            to = tgb[:, 3 * HALF:4 * HALF]
            nc.vector.scalar_tensor_tensor(out=ab, in0=ti, scalar=1.0, in1=tg_,
                                           op0=OP.add, op1=OP.mult)
            nc.vector.scalar_tensor_tensor(out=bb, in0=tf, scalar=1.0, in1=st,
                                           op0=OP.add, op1=OP.mult)
            nc.vector.scalar_tensor_tensor(out=st, in0=bb, scalar=0.5, in1=ab,
                                           op0=OP.mult, op1=OP.add)
            nc.scalar.activation(tcb, st, AF.Tanh, scale=c_half)
            nc.vector.scalar_tensor_tensor(out=htile, in0=to, scalar=1.0, in1=tcb,
                                           op0=OP.add, op1=OP.mult)

        for t in range(TS):
            xo_t = xo[t % 3]
            nc.sync.dma_start(out=xo_t[0:1, :], in_=xt[t:t + 1, 0:HALF])
            nc.sync.dma_start(out=xo_t[64:65, :], in_=xt[t:t + 1, HALF:BN])

            l0ps = psl0.tile([128, 4 * HALF], dt.float32, tag="l0")
            lstm_matmuls(l0ps, WH0, ht0, layer1=False)
            nc.scalar.activation(tg0, l0ps, AF.Tanh)
            cell_update(tg0, s0t, a0, b0, tc0, ht0)

            l1ps = psl1.tile([128, 4 * HALF], dt.float32, tag="l1")
            lstm_matmuls(l1ps, WI1, ht0, layer1=True)
            nc.scalar.activation(tg1, l1ps, AF.Tanh)
            cell_update(tg1, s1t, a1, b1t_, tc1, ht1)

            hps = psl1.tile([2, BN], dt.float32, tag="l1", name="hps")
            nc.tensor.matmul(hps[0:2, 0:HALF], lhsT=HD[0:64, 0:2],
                             rhs=ht1[0:64, :], start=True, stop=True,
                             tile_position=(0, 0))
            nc.tensor.matmul(hps[0:2, HALF:BN], lhsT=HD[64:128, 0:2],
                             rhs=ht1[64:128, :], start=True, stop=True,
                             tile_position=(64, 0))
            ms = musig_sb[t % 2]
            nc.vector.tensor_copy(ms, hps)
            nc.sync.dma_start(out=musig_d[t], in_=ms)

        # ---- post-pass ----------------------------------------------------
        for c in range(NCHUNK):
            b_, n0 = c // 2, (c % 2) * 128
            fc, mv, sd, isd = full_c[c], mv_c[c], stdev_c[c], istd_c[c]

            mu_tf = work.tile([128, 128], dt.bfloat16, tag="mu_tf")
            sg_tf = work.tile([128, 128], dt.bfloat16, tag="sg_tf")
            nc.sync.dma_start_transpose(out=mu_tf, in_=musig_d[:, 0, c * 128:(c + 1) * 128])
            nc.sync.dma_start_transpose(out=sg_tf, in_=musig_d[:, 1, c * 128:(c + 1) * 128])
            mu_t = mu_tf[:, 0:TS]
            sg_t = sg_tf[:, 0:TS]

            eps_c = work.tile([128, TS], dt.float32, tag="eps_c")
            nc.sync.dma_start(out=eps_c,
                              in_=epsin[b_, :, n0:n0 + 128].rearrange("t n -> n t"))
            mk = work.tile([128, TS], dt.float32, tag="mk")
            nc.sync.dma_start(out=mk[:, 0:LIN - 1],
                              in_=hmask[b_, 1:LIN, n0:n0 + 128].rearrange("t n -> n t"))
            nc.sync.dma_start(out=mk[:, LIN - 1:TS],
                              in_=fmask[b_, :, n0:n0 + 128].rearrange("t n -> n t"))

            # sigma = softplus(sg + sigma_b) + 1e-6  (stable exp/log form)
            ab_ = work.tile([128, TS], dt.float32, tag="ab_")
            nc.scalar.activation(ab_, sg_t, AF.Abs, bias=c_sigb)
            ex_ = work.tile([128, TS], dt.float32, tag="ex_")
            nc.scalar.activation(ex_, ab_, AF.Exp, scale=c_neg1)
            ln_ = work.tile([128, TS], dt.float32, tag="ln_")
            nc.scalar.activation(ln_, ex_, AF.Ln, bias=1.0)
            rl_ = work.tile([128, TS], dt.float32, tag="rl_")
            nc.vector.tensor_scalar(out=rl_, in0=sg_t, scalar1=sigma_b,
                                    scalar2=0.0, op0=OP.add, op1=OP.max)
            sig = work.tile([128, TS], dt.float32, tag="sig")
            nc.vector.scalar_tensor_tensor(out=sig, in0=ln_, scalar=1e-6, in1=rl_,
                                           op0=OP.add, op1=OP.add)

            # preds = ((mu+mu_b) + sigma*eps)*stdev + means, masked
            m1 = work.tile([128, TS], dt.float32, tag="m1")
            nc.vector.tensor_tensor(out=m1, in0=sig, in1=eps_c, op=OP.mult)
            m2 = work.tile([128, TS], dt.float32, tag="m2")
            nc.vector.scalar_tensor_tensor(out=m2, in0=mu_t, scalar=mu_b, in1=m1,
                                           op0=OP.add, op1=OP.add)
            m3 = work.tile([128, TS], dt.float32, tag="m3")
            nc.vector.tensor_scalar(out=m3, in0=m2, scalar1=sd, scalar2=mv[:, 0:1],
                                    op0=OP.mult, op1=OP.add)
            pr = work.tile([128, TS], dt.float32, tag="pr")
            nc.vector.tensor_tensor(out=pr, in0=m3, in1=mk, op=OP.mult)
            nc.sync.dma_start(out=o_preds[b_, :, n0:n0 + 128].rearrange("t n -> n t"),
                              in_=pr)

            # reals = full[:, 1:] * mask
            rr = work.tile([128, TS], dt.float32, tag="rr")
            nc.vector.tensor_tensor(out=rr, in0=fc[:, 1:T], in1=mk, op=OP.mult)
            nc.sync.dma_start(out=o_reals[b_, :, n0:n0 + 128].rearrange("t n -> n t"),
                              in_=rr)

            # mus = (mu+mu_b)*stdev + means
            u1 = work.tile([128, TS], dt.float32, tag="u1")
            nc.vector.tensor_scalar(out=u1, in0=mu_t, scalar1=mu_b, scalar2=None,
                                    op0=OP.add)
            u2 = work.tile([128, TS], dt.float32, tag="u2")
            nc.vector.tensor_scalar(out=u2, in0=u1, scalar1=sd, scalar2=mv[:, 0:1],
                                    op0=OP.mult, op1=OP.add)
            nc.sync.dma_start(out=o_mus[b_, :, n0:n0 + 128].rearrange("t n -> n t"),
                              in_=u2)

            # sigmas = sigma*stdev + means
            v1 = work.tile([128, TS], dt.float32, tag="v1")
            nc.vector.tensor_scalar(out=v1, in0=sig, scalar1=sd, scalar2=mv[:, 0:1],
                                    op0=OP.mult, op1=OP.add)
            nc.sync.dma_start(out=o_sigs[b_, :, n0:n0 + 128].rearrange("t n -> n t"),
                              in_=v1)

            # mask passthrough
            nc.sync.dma_start(out=o_mask[b_, :, n0:n0 + 128].rearrange("t n -> n t"),
                              in_=mk)

    nc.finalize()
    return nc


def kernel(**inputs):
    import os
    from concourse.bass_utils import run_bass_kernel_spmd

    f32 = np.float32
    packs = _pack_weights(inputs)

    key = "nc"
    if key not in _cache:
        _cache[key] = _build(packs["mu_b"], packs["sigma_b"])
    nc = _cache[key]

    hist = np.ascontiguousarray(np.asarray(inputs["history_data"], f32)[..., 0])
    fut = np.ascontiguousarray(np.asarray(inputs["future_data"], f32)[..., 0])
    hm = np.ascontiguousarray(np.asarray(inputs["history_mask"], f32))
    fm = np.ascontiguousarray(np.asarray(inputs["future_mask"], f32))
    eps = np.ascontiguousarray(np.asarray(inputs["eps"], f32)[..., 0])

    in_maps = []
    for c in range(NCORES):
        b0, b1 = c * BL, (c + 1) * BL
        m = {
            "hist": hist[b0:b1], "fut": fut[b0:b1],
            "hmask": hm[b0:b1], "fmask": fm[b0:b1], "epsin": eps[b0:b1],
        }
        for k in ("WH0", "WX0", "WI1", "WH1", "B1T", "HD", "ID", "XOINIT"):
            m[k] = packs[k]
        in_maps.append(m)

    kres = run_bass_kernel_spmd(nc, in_maps, list(range(NCORES)),
                                trace=bool(os.environ.get("KERNEL_TRACE")))
    _cache["last"] = kres
    res = kres.results

    def gather(name):
        full = np.concatenate([res[c][name] for c in range(NCORES)], axis=0)
        return full.reshape(B, TS, N, 1).astype(f32)

    return (gather("preds"), gather("reals"), gather("musv"),
            gather("sigmasv"), gather("maskv"))


# revision 19
# speedup vs baseline: 1.5398x; 1.5398x over previous
"""DeepAR Trainium2 Bass kernel.

Strategy (hardcoded from spec nn_DeepAR_90374701843258):
  B=32, LIN=96, LOUT=24, N=256, E=32, H=64, T-1=119 steps, 8 cores.
  Data-parallel over B: 4 batch rows per core -> per-core batch BN=1024.
  Layout: "folded" [128, 512] tiles everywhere: partition p<64 = H-unit p of
  batch half 0 (cols 0:512 of the 1024 batch), p>=64 = H-unit p-64 of half 1.

  Algebra:
   - embedding + layer0 input proj collapse to rank-1: pre0 = x*w_eff + b_eff
     (w_eff = Wih0 @ embed_W), injected as extra contraction rows in the
     recurrent matmul (x and ones rows live in the xo tile).
   - sigmoid(z) = (tanh(z/2)+1)/2: i,f,o gate weights are pre-scaled by 0.5 so
     ALL four gates use one Tanh activation pass. h is stored as Hs=2h (the
     next-layer weights absorb the 0.5), cell state as s=2c.
     s' = 0.5*(tf+1)*s + (ti+1)*tg ; Hs = (to+1)*tanh(0.5*s')
     -> 4 fused scalar_tensor_tensor DVE ops per layer.
"""

import numpy as np

B, LIN, LOUT, N, E, H = 32, 96, 24, 256, 32, 64
T = LIN + LOUT
TS = T - 1            # 119
NCORES = 8
BL = B // NCORES      # 4
BN = BL * N           # 1024
HALF = 512
NCHUNK = BN // 128    # 8

_cache = {}


def _pack_weights(inp):
    """Host-side weight prep (tiny arrays). Returns dict of np arrays."""
    import ml_dtypes
    bf16 = ml_dtypes.bfloat16
    f32 = np.float32

    Wih0, Whh0 = inp["Wih0"].astype(f32), inp["Whh0"].astype(f32)
    Wih1, Whh1 = inp["Wih1"].astype(f32), inp["Whh1"].astype(f32)
    w_eff = (Wih0 @ inp["embed_W"].astype(f32))[:, 0]
    b_eff = Wih0 @ inp["embed_b"].astype(f32) + inp["bih0"] + inp["bhh0"]
    b1 = (inp["bih1"] + inp["bhh1"]).astype(f32)

    sc = np.ones(4 * H, f32)
    sc[: 2 * H] = 0.5      # i, f
    sc[3 * H:] = 0.5       # o
    habs = 0.5             # h stored as 2h

    def two_copies(top):
        w = np.zeros((128, 4 * H), f32)
        w[0:64] = top
        w[64:128] = top
        return w

    WH0 = two_copies(Whh0.T * (habs * sc)[None, :])
    WI1 = two_copies(Wih1.T * (habs * sc)[None, :])
    WH1 = two_copies(Whh1.T * (habs * sc)[None, :])

    WX0 = np.zeros((128, 4 * H), f32)
    WX0[64] = w_eff * sc
    WX0[65] = b_eff * sc
    WX0[0] = w_eff * sc
    WX0[1] = b_eff * sc

    B1T = np.zeros((128, 4 * H), f32)
    B1T[32] = b1 * sc

    HD = np.zeros((128, 2), f32)
    HD[0:64, 0] = inp["mu_W"].astype(f32)[0] * habs
    HD[0:64, 1] = inp["sigma_W"].astype(f32)[0] * habs
    HD[64:128] = HD[0:64]

    ID = np.eye(128, dtype=f32)

    return {
        "WH0": WH0.astype(bf16), "WX0": WX0.astype(bf16),
        "WI1": WI1.astype(bf16), "WH1": WH1.astype(bf16),
        "B1T": B1T.astype(bf16), "HD": HD.astype(bf16),
        "ID": ID.astype(bf16),
        "ONESBIG": np.ones((1, TS * HALF), f32).astype(bf16),
        "IDF": np.eye(128, dtype=f32),
        "mu_b": float(inp["mu_b"][0]), "sigma_b": float(inp["sigma_b"][0]),
    }


def _build(mu_b, sigma_b):
    """Build the per-core bass program (SPMD: identical on all cores)."""
    from contextlib import ExitStack
    import concourse.bass as bass
    import concourse.mybir as mybir
    import concourse.tile as tile
    from concourse import bacc

    dt = mybir.dt
    AF = mybir.ActivationFunctionType
    OP = mybir.AluOpType

    nc = bacc.Bacc()

    # ---- I/O declarations -------------------------------------------------
    hist = nc.declare_dram_parameter("hist", [BL, LIN, N], dt.float32, isOutput=False)
    fut = nc.declare_dram_parameter("fut", [BL, LOUT, N], dt.float32, isOutput=False)
    hmask = nc.declare_dram_parameter("hmask", [BL, LIN, N], dt.float32, isOutput=False)
    fmask = nc.declare_dram_parameter("fmask", [BL, LOUT, N], dt.float32, isOutput=False)
    epsin = nc.declare_dram_parameter("epsin", [BL, TS, N], dt.float32, isOutput=False)
    wWH0 = nc.declare_dram_parameter("WH0", [128, 4 * H], dt.bfloat16, isOutput=False)
    wWX0 = nc.declare_dram_parameter("WX0", [128, 4 * H], dt.bfloat16, isOutput=False)
    wWI1 = nc.declare_dram_parameter("WI1", [128, 4 * H], dt.bfloat16, isOutput=False)
    wWH1 = nc.declare_dram_parameter("WH1", [128, 4 * H], dt.bfloat16, isOutput=False)
    wB1T = nc.declare_dram_parameter("B1T", [128, 4 * H], dt.bfloat16, isOutput=False)
    wHD = nc.declare_dram_parameter("HD", [128, 2], dt.bfloat16, isOutput=False)
    wID = nc.declare_dram_parameter("ID", [128, 128], dt.bfloat16, isOutput=False)
    wONB = nc.declare_dram_parameter("ONESBIG", [1, TS * HALF], dt.bfloat16, isOutput=False)
    wIDF = nc.declare_dram_parameter("IDF", [128, 128], dt.float32, isOutput=False)

    o_preds = nc.declare_dram_parameter("preds", [BL, TS, N], dt.float32, isOutput=True)
    o_reals = nc.declare_dram_parameter("reals", [BL, TS, N], dt.float32, isOutput=True)
    o_mus = nc.declare_dram_parameter("musv", [BL, TS, N], dt.float32, isOutput=True)
    o_sigs = nc.declare_dram_parameter("sigmasv", [BL, TS, N], dt.float32, isOutput=True)
    o_mask = nc.declare_dram_parameter("maskv", [BL, TS, N], dt.float32, isOutput=True)

    musig_d = nc.dram_tensor("musig", [2, 128, BN], dt.bfloat16)
    xs_d = nc.dram_tensor("xsd", [TS, BN], dt.bfloat16)

    with ExitStack() as ctx:
        tc = ctx.enter_context(tile.TileContext(nc))
        persist = ctx.enter_context(tc.tile_pool(name="persist", bufs=1))
        work = ctx.enter_context(tc.tile_pool(name="work", bufs=3))
        psl0 = ctx.enter_context(tc.tile_pool(name="psl0", bufs=1, space="PSUM"))
        psl1 = ctx.enter_context(tc.tile_pool(name="psl1", bufs=1, space="PSUM"))

        # ---- constants / weights into SBUF -------------------------------
        WH0 = persist.tile([128, 4 * H], dt.bfloat16, tag="WH0")
        WX0 = persist.tile([128, 4 * H], dt.bfloat16, tag="WX0")
        WI1 = persist.tile([128, 4 * H], dt.bfloat16, tag="WI1")
        WH1 = persist.tile([128, 4 * H], dt.bfloat16, tag="WH1")
        B1T = persist.tile([128, 4 * H], dt.bfloat16, tag="B1T")
        HD = persist.tile([128, 2], dt.bfloat16, tag="HD")
        ID = persist.tile([128, 128], dt.bfloat16, tag="ID")
        IDF = persist.tile([128, 128], dt.float32, tag="IDF")
        for t_, d_ in [(WH0, wWH0), (WX0, wWX0), (WI1, wWI1), (WH1, wWH1),
                       (B1T, wB1T), (HD, wHD), (ID, wID), (IDF, wIDF)]:
            nc.sync.dma_start(out=t_[:], in_=d_[:])

        c_half = persist.tile([128, 1], dt.float32, tag="c_half")
        nc.vector.memset(c_half, 0.5)
        c_neg1 = persist.tile([128, 1], dt.float32, tag="c_neg1")
        nc.vector.memset(c_neg1, -1.0)
        c_sigb = persist.tile([128, 1], dt.float32, tag="c_sigb")
        nc.vector.memset(c_sigb, sigma_b)

        # ---- persistent state tiles ---------------------------------------
        ht0 = persist.tile([128, HALF], dt.bfloat16, tag="ht0")   # Hs0 = 2*h0
        ht1 = persist.tile([128, HALF], dt.bfloat16, tag="ht1")   # Hs1 = 2*h1
        s0t = persist.tile([128, HALF], dt.bfloat16, tag="s0t")   # s0 = 2*c0
        s1t = persist.tile([128, HALF], dt.bfloat16, tag="s1t")   # s1 = 2*c1
        for t_ in (ht0, ht1, s0t, s1t):
            nc.vector.memset(t_, 0.0)

        tg0 = persist.tile([128, 4 * HALF], dt.bfloat16, tag="tg0")
        tg1 = persist.tile([128, 4 * HALF], dt.bfloat16, tag="tg1")
        a0 = persist.tile([128, HALF], dt.bfloat16, tag="a0")
        b0 = persist.tile([128, HALF], dt.bfloat16, tag="b0")
        a1 = persist.tile([128, HALF], dt.bfloat16, tag="a1")
        b1t_ = persist.tile([128, HALF], dt.bfloat16, tag="b1t_")
        tc0 = persist.tile([128, HALF], dt.bfloat16, tag="tc0")
        tc1 = persist.tile([128, HALF], dt.bfloat16, tag="tc1")

        xall = persist.tile([128, TS * HALF], dt.bfloat16, tag="xall")
        for p_ in (1, 32, 65):
            nc.sync.dma_start(out=xall[p_:p_ + 1, :], in_=wONB[:])

        ms8 = [persist.tile([2, 8 * BN], dt.bfloat16, tag=f"ms8{i}", name=f"ms8{i}")
               for i in range(2)]

        xt = persist.tile([TS, BN], dt.bfloat16, tag="xt")

        # stats + raw data per chunk (kept for the post-pass)
        full_c, mv_c, stdev_c, istd_c = [], [], [], []

        # ---- pre-pass: stats, normalize, transpose x ----------------------
        for c in range(NCHUNK):
            b_, n0 = c // 2, (c % 2) * 128
            raw = work.tile([T, 128], dt.float32, tag="raw")
            nc.sync.dma_start(out=raw[0:LIN, :], in_=hist[b_, :, n0:n0 + 128])
            nc.sync.dma_start(out=raw[LIN:T, :], in_=fut[b_, :, n0:n0 + 128])
            fpt = psl1.tile([128, T], dt.float32, tag="l1", name="fpt")
            nc.tensor.transpose(fpt, raw, IDF[0:T, 0:T])
            fc = persist.tile([128, T], dt.float32, tag=f"full{c}")
            nc.vector.tensor_copy(fc, fpt)
            st6 = work.tile([128, 6], dt.float32, tag="st6")
            mv = persist.tile([128, 2], dt.float32, tag=f"mv{c}")
            nc.vector.bn_stats(out=st6, in_=fc[:, 0:LIN])
            nc.vector.bn_aggr(out=mv, in_=st6)
            # stdev = sqrt(var+1e-5) with one Newton refinement (Sqrt table is
            # low precision); istd = 1/stdev via DVE reciprocal.
            veps = work.tile([128, 1], dt.float32, tag="veps")
            nc.vector.tensor_scalar(out=veps, in0=mv[:, 1:2], scalar1=1e-5,
                                    scalar2=None, op0=OP.add)
            y0 = work.tile([128, 1], dt.float32, tag="y0")
            nc.scalar.activation(y0, veps, AF.Sqrt)
            r0 = work.tile([128, 1], dt.float32, tag="r0")
            nc.vector.reciprocal(r0, y0)
            yy = work.tile([128, 1], dt.float32, tag="yy")
            nc.vector.tensor_tensor(out=yy, in0=y0, in1=y0, op=OP.mult)
            e_ = work.tile([128, 1], dt.float32, tag="e_")
            nc.vector.tensor_tensor(out=e_, in0=veps, in1=yy, op=OP.subtract)
            d_ = work.tile([128, 1], dt.float32, tag="d_")
            nc.vector.scalar_tensor_tensor(out=d_, in0=e_, scalar=0.5, in1=r0,
                                           op0=OP.mult, op1=OP.mult)
            sd = persist.tile([128, 1], dt.float32, tag=f"sd{c}")
            nc.vector.tensor_tensor(out=sd, in0=y0, in1=d_, op=OP.add)
            isd = persist.tile([128, 1], dt.float32, tag=f"isd{c}")
            nc.vector.reciprocal(isd, sd)
            full_c.append(fc); mv_c.append(mv); stdev_c.append(sd); istd_c.append(isd)

            # normalized x for steps 0..118, bf16
            xn = work.tile([128, TS], dt.bfloat16, tag="xn")
            nc.vector.tensor_scalar(out=xn, in0=fc[:, 0:TS], scalar1=mv[:, 0:1],
                                    scalar2=isd, op0=OP.subtract, op1=OP.mult)
            # transpose to [TS, 128] and write into xt with halves swapped
            pt = psl0.tile([TS, 128], dt.bfloat16, tag="l0", name="pt")
            nc.tensor.transpose(pt, xn, ID)
            xtcol = (1 - c // 4) * HALF + (c % 4) * 128
            nc.vector.tensor_copy(xt[:, xtcol:xtcol + 128], pt)

        # stage xt to DRAM, then load x rows into xall partitions 0 / 64
        nc.sync.dma_start(out=xs_d[:], in_=xt[:])
        nc.sync.dma_start(
            out=xall[0:1, :].rearrange("p (t b) -> p t b", b=HALF),
            in_=xs_d[None, :, 0:HALF])
        nc.sync.dma_start(
            out=xall[64:65, :].rearrange("p (t b) -> p t b", b=HALF),
            in_=xs_d[None, :, HALF:BN])

        # ---- main loop ----------------------------------------------------
        GS = [slice(X * 64, (X + 1) * 64) for X in range(4)]  # gate cols in W packs

        def lstm_matmuls(ps, Wh, hsrc, layer1, ts_):
            for X in range(4):
                psX = ps[:, X * HALF:(X + 1) * HALF]
                g = GS[X]
                # half 0
                nc.tensor.matmul(psX[0:64, :], lhsT=Wh[0:64, g], rhs=hsrc[0:64, :],
                                 start=True, stop=False, tile_position=(0, 0))
                if layer1:
                    nc.tensor.matmul(psX[0:64, :], lhsT=WH1[0:64, g],
                                     rhs=ht1[0:64, :], start=False, stop=False,
                                     tile_position=(0, 0))
                    nc.tensor.matmul(psX[0:64, :], lhsT=B1T[32:33, g],
                                     rhs=xall[32:33, ts_:ts_ + HALF],
                                     start=False, stop=True,
                                     tile_position=(32, 0))
                else:
                    nc.tensor.matmul(psX[0:64, :], lhsT=WX0[64:66, g],
                                     rhs=xall[64:66, ts_:ts_ + HALF],
                                     start=False, stop=True,
                                     tile_position=(64, 0))
                # half 1
                nc.tensor.matmul(psX[64:128, :], lhsT=Wh[64:128, g],
                                 rhs=hsrc[64:128, :], start=True, stop=False,
                                 tile_position=(64, 64))
                if layer1:
                    nc.tensor.matmul(psX[64:128, :], lhsT=WH1[64:128, g],
                                     rhs=ht1[64:128, :], start=False, stop=False,
                                     tile_position=(64, 64))
                    nc.tensor.matmul(psX[64:128, :], lhsT=B1T[32:33, g],
                                     rhs=xall[32:33, ts_:ts_ + HALF],
                                     start=False, stop=True,
                                     tile_position=(32, 64))
                else:
                    nc.tensor.matmul(psX[64:128, :], lhsT=WX0[0:2, g],
                                     rhs=xall[0:2, ts_:ts_ + HALF],
                                     start=False, stop=True,
                                     tile_position=(0, 64))

        def cell_update(tgb, st, ab, bb, tcb, htile):
            ti = tgb[:, 0:HALF]
            tf = tgb[:, HALF:2 * HALF]
            tg_ = tgb[:, 2 * HALF:3 * HALF]
            to = tgb[:, 3 * HALF:4 * HALF]
            nc.vector.scalar_tensor_tensor(out=ab, in0=ti, scalar=1.0, in1=tg_,
                                           op0=OP.add, op1=OP.mult)
            nc.vector.scalar_tensor_tensor(out=bb, in0=tf, scalar=1.0, in1=st,
                                           op0=OP.add, op1=OP.mult)
            nc.vector.scalar_tensor_tensor(out=st, in0=bb, scalar=0.5, in1=ab,
                                           op0=OP.mult, op1=OP.add)
            nc.scalar.activation(tcb, st, AF.Tanh, scale=c_half)
            nc.vector.scalar_tensor_tensor(out=htile, in0=to, scalar=1.0, in1=tcb,
                                           op0=OP.add, op1=OP.mult)

        for t in range(TS):
            ts_ = t * HALF
            l0ps = psl0.tile([128, 4 * HALF], dt.float32, tag="l0")
            lstm_matmuls(l0ps, WH0, ht0, layer1=False, ts_=ts_)
            nc.scalar.activation(tg0, l0ps, AF.Tanh)
            cell_update(tg0, s0t, a0, b0, tc0, ht0)

            l1ps = psl1.tile([128, 4 * HALF], dt.float32, tag="l1")
            lstm_matmuls(l1ps, WI1, ht0, layer1=True, ts_=ts_)
            nc.scalar.activation(tg1, l1ps, AF.Tanh)
            cell_update(tg1, s1t, a1, b1t_, tc1, ht1)

            hps = psl1.tile([2, BN], dt.float32, tag="l1", name="hps")
            nc.tensor.matmul(hps[0:2, 0:HALF], lhsT=HD[0:64, 0:2],
                             rhs=ht1[0:64, :], start=True, stop=True,
                             tile_position=(0, 0))
            nc.tensor.matmul(hps[0:2, HALF:BN], lhsT=HD[64:128, 0:2],
                             rhs=ht1[64:128, :], start=True, stop=True,
                             tile_position=(64, 0))
            ring = ms8[(t // 8) % 2]
            nc.vector.tensor_copy(ring[:, (t % 8) * BN:(t % 8 + 1) * BN], hps)
            if t % 8 == 7 or t == TS - 1:
                k0 = t - (t % 8)
                nw = t - k0 + 1
                nc.sync.dma_start(
                    out=musig_d[:, k0:t + 1, :],
                    in_=ring[:, 0:nw * BN].rearrange("h (s b) -> h s b", b=BN))

        # ---- post-pass ----------------------------------------------------
        for c in range(NCHUNK):
            b_, n0 = c // 2, (c % 2) * 128
            fc, mv, sd, isd = full_c[c], mv_c[c], stdev_c[c], istd_c[c]

            mu_tf = work.tile([128, 128], dt.bfloat16, tag="mu_tf")
            sg_tf = work.tile([128, 128], dt.bfloat16, tag="sg_tf")
            nc.sync.dma_start_transpose(out=mu_tf, in_=musig_d[0, :, c * 128:(c + 1) * 128])
            nc.sync.dma_start_transpose(out=sg_tf, in_=musig_d[1, :, c * 128:(c + 1) * 128])
            mu_t = mu_tf[:, 0:TS]
            sg_t = sg_tf[:, 0:TS]

            eps_c = work.tile([128, TS], dt.float32, tag="eps_c")
            nc.sync.dma_start(out=eps_c,
                              in_=epsin[b_, :, n0:n0 + 128].rearrange("t n -> n t"))
            mk = work.tile([128, TS], dt.float32, tag="mk")
            nc.sync.dma_start(out=mk[:, 0:LIN - 1],
                              in_=hmask[b_, 1:LIN, n0:n0 + 128].rearrange("t n -> n t"))
            nc.sync.dma_start(out=mk[:, LIN - 1:TS],
                              in_=fmask[b_, :, n0:n0 + 128].rearrange("t n -> n t"))

            # sigma = softplus(sg + sigma_b) + 1e-6  (stable exp/log form)
            ab_ = work.tile([128, TS], dt.float32, tag="ab_")
            nc.scalar.activation(ab_, sg_t, AF.Abs, bias=c_sigb)
            ex_ = work.tile([128, TS], dt.float32, tag="ex_")
            nc.scalar.activation(ex_, ab_, AF.Exp, scale=c_neg1)
            ln_ = work.tile([128, TS], dt.float32, tag="ln_")
            nc.scalar.activation(ln_, ex_, AF.Ln, bias=1.0)
            rl_ = work.tile([128, TS], dt.float32, tag="rl_")
            nc.vector.tensor_scalar(out=rl_, in0=sg_t, scalar1=sigma_b,
                                    scalar2=0.0, op0=OP.add, op1=OP.max)
            sig = work.tile([128, TS], dt.float32, tag="sig")
            nc.vector.scalar_tensor_tensor(out=sig, in0=ln_, scalar=1e-6, in1=rl_,
                                           op0=OP.add, op1=OP.add)

            # preds = ((mu+mu_b) + sigma*eps)*stdev + means, masked
            m1 = work.tile([128, TS], dt.float32, tag="m1")
            nc.vector.tensor_tensor(out=m1, in0=sig, in1=eps_c, op=OP.mult)
            m2 = work.tile([128, TS], dt.float32, tag="m2")
            nc.vector.scalar_tensor_tensor(out=m2, in0=mu_t, scalar=mu_b, in1=m1,
                                           op0=OP.add, op1=OP.add)
            m3 = work.tile([128, TS], dt.float32, tag="m3")
            nc.vector.tensor_scalar(out=m3, in0=m2, scalar1=sd, scalar2=mv[:, 0:1],
                                    op0=OP.mult, op1=OP.add)
            pr = work.tile([128, TS], dt.float32, tag="pr")
            nc.vector.tensor_tensor(out=pr, in0=m3, in1=mk, op=OP.mult)

            rr = work.tile([128, TS], dt.float32, tag="rr")
            nc.vector.tensor_tensor(out=rr, in0=fc[:, 1:T], in1=mk, op=OP.mult)

            u1 = work.tile([128, TS], dt.float32, tag="u1")
            nc.vector.tensor_scalar(out=u1, in0=mu_t, scalar1=mu_b, scalar2=None,
                                    op0=OP.add)
            u2 = work.tile([128, TS], dt.float32, tag="u2")
            nc.vector.tensor_scalar(out=u2, in0=u1, scalar1=sd, scalar2=mv[:, 0:1],
                                    op0=OP.mult, op1=OP.add)

            v1 = work.tile([128, TS], dt.float32, tag="v1")
            nc.vector.tensor_scalar(out=v1, in0=sig, scalar1=sd, scalar2=mv[:, 0:1],
                                    op0=OP.mult, op1=OP.add)

            # transpose each output [n,t] -> [t,n] on PE, then contiguous DMA
            for src_t, odram in ((pr, o_preds), (rr, o_reals), (u2, o_mus),
                                 (v1, o_sigs), (mk, o_mask)):
                tps = psl0.tile([TS, 128], dt.float32, tag="l0", name="tps")
                nc.tensor.transpose(tps, src_t, IDF)
                osb = work.tile([TS, 128], dt.float32, tag="osb", bufs=4)
                nc.vector.tensor_copy(osb, tps)
                nc.sync.dma_start(out=odram[b_, :, n0:n0 + 128], in_=osb)

    nc.finalize()
    return nc


def kernel(**inputs):
    import os
    from concourse.bass_utils import run_bass_kernel_spmd

    f32 = np.float32
    packs = _pack_weights(inputs)

    key = "nc"
    if key not in _cache:
        _cache[key] = _build(packs["mu_b"], packs["sigma_b"])
    nc = _cache[key]

    hist = np.ascontiguousarray(np.asarray(inputs["history_data"], f32)[..., 0])
    fut = np.ascontiguousarray(np.asarray(inputs["future_data"], f32)[..., 0])
    hm = np.ascontiguousarray(np.asarray(inputs["history_mask"], f32))
    fm = np.ascontiguousarray(np.asarray(inputs["future_mask"], f32))
    eps = np.ascontiguousarray(np.asarray(inputs["eps"], f32)[..., 0])

    in_maps = []
    for c in range(NCORES):
        b0, b1 = c * BL, (c + 1) * BL
        m = {
            "hist": hist[b0:b1], "fut": fut[b0:b1],
            "hmask": hm[b0:b1], "fmask": fm[b0:b1], "epsin": eps[b0:b1],
        }
        for k in ("WH0", "WX0", "WI1", "WH1", "B1T", "HD", "ID", "ONESBIG", "IDF"):
            m[k] = packs[k]
        in_maps.append(m)

    kres = run_bass_kernel_spmd(nc, in_maps, list(range(NCORES)),
                                trace=bool(os.environ.get("KERNEL_TRACE")))
    _cache["last"] = kres
    res = kres.results

    def gather(name):
        full = np.concatenate([res[c][name] for c in range(NCORES)], axis=0)
        return full.reshape(B, TS, N, 1).astype(f32)

    return (gather("preds"), gather("reals"), gather("musv"),
            gather("sigmasv"), gather("maskv"))


# revision 20
# speedup vs baseline: 2.5117x; 1.6312x over previous
"""DeepAR Trainium2 Bass kernel.

Strategy (hardcoded from spec nn_DeepAR_90374701843258):
  B=32, LIN=96, LOUT=24, N=256, E=32, H=64, T-1=119 steps, 8 cores.
  Data-parallel over B: 4 batch rows per core -> per-core batch BN=1024.
  Layout: "folded" [128, 512] tiles everywhere: partition p<64 = H-unit p of
  batch half 0 (cols 0:512 of the 1024 batch), p>=64 = H-unit p-64 of half 1.

  Algebra:
   - embedding + layer0 input proj collapse to rank-1: pre0 = x*w_eff + b_eff
     (w_eff = Wih0 @ embed_W), injected as extra contraction rows in the
     recurrent matmul (x and ones rows live in the xo tile).
   - sigmoid(z) = (tanh(z/2)+1)/2: i,f,o gate weights are pre-scaled by 0.5 so
     ALL four gates use one Tanh activation pass. h is stored as Hs=2h (the
     next-layer weights absorb the 0.5), cell state as s=2c.
     s' = 0.5*(tf+1)*s + (ti+1)*tg ; Hs = (to+1)*tanh(0.5*s')
     -> 4 fused scalar_tensor_tensor DVE ops per layer.
"""

import numpy as np

B, LIN, LOUT, N, E, H = 32, 96, 24, 256, 32, 64
T = LIN + LOUT
TS = T - 1            # 119
NCORES = 8
BL = B // NCORES      # 4
BN = BL * N           # 1024
HALF = 512
NCHUNK = BN // 128    # 8

_cache = {}


def _pack_weights(inp):
    """Host-side weight prep (tiny arrays). Returns dict of np arrays.

    Block-diagonal stationary layout: each gate's matmul processes BOTH folded
    batch halves in one K=128 matmul with lhsT = diag(W_X^T, W_X^T).
    Gates i,f,g natural (real sigmoid/tanh on ACT); o-gate pre-scaled by 0.5
    (sigmoid(o) = 0.5*tanh(o/2)+0.5 computed on DVE). h and c natural.
    """
    import ml_dtypes
    bf16 = ml_dtypes.bfloat16
    f32 = np.float32

    Wih0, Whh0 = inp["Wih0"].astype(f32), inp["Whh0"].astype(f32)
    Wih1, Whh1 = inp["Wih1"].astype(f32), inp["Whh1"].astype(f32)
    w_eff = (Wih0 @ inp["embed_W"].astype(f32))[:, 0]
    b_eff = Wih0 @ inp["embed_b"].astype(f32) + inp["bih0"] + inp["bhh0"]
    b1 = (inp["bih1"] + inp["bhh1"]).astype(f32)

    sc = np.ones(4 * H, f32)
    sc[3 * H:] = 0.5       # o-gate only

    def blockdiag(Wm):
        # Wm [4H, H]; returns [128, 4*128]
        out = np.zeros((128, 4 * 128), f32)
        for X in range(4):
            wt = (Wm[X * H:(X + 1) * H].T * sc[X * H:(X + 1) * H][None, :])
            out[0:64, X * 128:X * 128 + 64] = wt
            out[64:128, X * 128 + 64:(X + 1) * 128] = wt
        return out

    WH0 = blockdiag(Whh0)
    WI1 = blockdiag(Wih1)
    WH1 = blockdiag(Whh1)

    WX0 = np.zeros((128, 4 * 128), f32)
    for X in range(4):
        we = w_eff[X * H:(X + 1) * H] * sc[X * H:(X + 1) * H]
        be = b_eff[X * H:(X + 1) * H] * sc[X * H:(X + 1) * H]
        WX0[0, X * 128 + 64:(X + 1) * 128] = we   # x half1 -> out parts 64:128
        WX0[1, X * 128 + 64:(X + 1) * 128] = be
        WX0[64, X * 128:X * 128 + 64] = we        # x half0 -> out parts 0:64
        WX0[65, X * 128:X * 128 + 64] = be

    B1T = np.zeros((128, 4 * 128), f32)
    for X in range(4):
        bb = b1[X * H:(X + 1) * H] * sc[X * H:(X + 1) * H]
        B1T[32, X * 128:X * 128 + 64] = bb
        B1T[32, X * 128 + 64:(X + 1) * 128] = bb

    HD = np.zeros((128, 4), f32)
    HD[0:64, 0] = inp["mu_W"].astype(f32)[0]
    HD[0:64, 1] = inp["sigma_W"].astype(f32)[0]
    HD[64:128, 2] = inp["mu_W"].astype(f32)[0]
    HD[64:128, 3] = inp["sigma_W"].astype(f32)[0]

    return {
        "WH0": WH0.astype(bf16), "WX0": WX0.astype(bf16),
        "WI1": WI1.astype(bf16), "WH1": WH1.astype(bf16),
        "B1T": B1T.astype(bf16), "HD": HD.astype(bf16),
        "ID": np.eye(128, dtype=f32).astype(bf16),
        "ONESBIG": np.ones((1, TS * HALF), f32).astype(bf16),
        "IDF": np.eye(128, dtype=f32),
        "mu_b": float(inp["mu_b"][0]), "sigma_b": float(inp["sigma_b"][0]),
    }


def _build(mu_b, sigma_b):
    """Build the per-core bass program (SPMD: identical on all cores)."""
    from contextlib import ExitStack
    import concourse.bass as bass
    import concourse.mybir as mybir
    import concourse.tile as tile
    from concourse import bacc

    dt = mybir.dt
    AF = mybir.ActivationFunctionType
    OP = mybir.AluOpType

    nc = bacc.Bacc()

    # ---- I/O declarations -------------------------------------------------
    hist = nc.declare_dram_parameter("hist", [BL, LIN, N], dt.float32, isOutput=False)
    fut = nc.declare_dram_parameter("fut", [BL, LOUT, N], dt.float32, isOutput=False)
    hmask = nc.declare_dram_parameter("hmask", [BL, LIN, N], dt.float32, isOutput=False)
    fmask = nc.declare_dram_parameter("fmask", [BL, LOUT, N], dt.float32, isOutput=False)
    epsin = nc.declare_dram_parameter("epsin", [BL, TS, N], dt.float32, isOutput=False)
    wWH0 = nc.declare_dram_parameter("WH0", [128, 512], dt.bfloat16, isOutput=False)
    wWX0 = nc.declare_dram_parameter("WX0", [128, 512], dt.bfloat16, isOutput=False)
    wWI1 = nc.declare_dram_parameter("WI1", [128, 512], dt.bfloat16, isOutput=False)
    wWH1 = nc.declare_dram_parameter("WH1", [128, 512], dt.bfloat16, isOutput=False)
    wB1T = nc.declare_dram_parameter("B1T", [128, 512], dt.bfloat16, isOutput=False)
    wHD = nc.declare_dram_parameter("HD", [128, 4], dt.bfloat16, isOutput=False)
    wID = nc.declare_dram_parameter("ID", [128, 128], dt.bfloat16, isOutput=False)
    wONB = nc.declare_dram_parameter("ONESBIG", [1, TS * HALF], dt.bfloat16, isOutput=False)
    wIDF = nc.declare_dram_parameter("IDF", [128, 128], dt.float32, isOutput=False)

    o_preds = nc.declare_dram_parameter("preds", [BL, TS, N], dt.float32, isOutput=True)
    o_reals = nc.declare_dram_parameter("reals", [BL, TS, N], dt.float32, isOutput=True)
    o_mus = nc.declare_dram_parameter("musv", [BL, TS, N], dt.float32, isOutput=True)
    o_sigs = nc.declare_dram_parameter("sigmasv", [BL, TS, N], dt.float32, isOutput=True)
    o_mask = nc.declare_dram_parameter("maskv", [BL, TS, N], dt.float32, isOutput=True)

    musig_d = nc.dram_tensor("musig", [4, 128, HALF], dt.bfloat16)
    xs_d = nc.dram_tensor("xsd", [TS, BN], dt.bfloat16)

    with ExitStack() as ctx:
        tc = ctx.enter_context(tile.TileContext(nc))
        persist = ctx.enter_context(tc.tile_pool(name="persist", bufs=1))
        work = ctx.enter_context(tc.tile_pool(name="work", bufs=3))
        psl0 = ctx.enter_context(tc.tile_pool(name="psl0", bufs=1, space="PSUM"))
        psl1 = ctx.enter_context(tc.tile_pool(name="psl1", bufs=1, space="PSUM"))

        # ---- constants / weights into SBUF -------------------------------
        WH0 = persist.tile([128, 512], dt.bfloat16, tag="WH0")
        WX0 = persist.tile([128, 512], dt.bfloat16, tag="WX0")
        WI1 = persist.tile([128, 512], dt.bfloat16, tag="WI1")
        WH1 = persist.tile([128, 512], dt.bfloat16, tag="WH1")
        B1T = persist.tile([128, 512], dt.bfloat16, tag="B1T")
        HD = persist.tile([128, 4], dt.bfloat16, tag="HD")
        ID = persist.tile([128, 128], dt.bfloat16, tag="ID")
        IDF = persist.tile([128, 128], dt.float32, tag="IDF")
        for t_, d_ in [(WH0, wWH0), (WX0, wWX0), (WI1, wWI1), (WH1, wWH1),
                       (B1T, wB1T), (HD, wHD), (ID, wID), (IDF, wIDF)]:
            nc.sync.dma_start(out=t_[:], in_=d_[:])

        c_half = persist.tile([128, 1], dt.float32, tag="c_half")
        nc.vector.memset(c_half, 0.5)
        c_neg1 = persist.tile([128, 1], dt.float32, tag="c_neg1")
        nc.vector.memset(c_neg1, -1.0)
        c_sigb = persist.tile([128, 1], dt.float32, tag="c_sigb")
        nc.vector.memset(c_sigb, sigma_b)

        # ---- persistent state tiles ---------------------------------------
        ht0 = persist.tile([128, HALF], dt.bfloat16, tag="ht0")
        ht1 = persist.tile([128, HALF], dt.bfloat16, tag="ht1")
        c0t = persist.tile([128, HALF], dt.bfloat16, tag="c0t")
        c1t = persist.tile([128, HALF], dt.bfloat16, tag="c1t")
        for t_ in (ht0, ht1, c0t, c1t):
            nc.vector.memset(t_, 0.0)

        tg0 = persist.tile([128, 4 * HALF], dt.bfloat16, tag="tg0")
        tg1 = persist.tile([128, 4 * HALF], dt.bfloat16, tag="tg1")
        a0 = persist.tile([128, HALF], dt.bfloat16, tag="a0")
        b0 = persist.tile([128, HALF], dt.bfloat16, tag="b0")
        a1 = persist.tile([128, HALF], dt.bfloat16, tag="a1")
        b1t_ = persist.tile([128, HALF], dt.bfloat16, tag="b1t_")
        tc0 = persist.tile([128, HALF], dt.bfloat16, tag="tc0")
        tc1 = persist.tile([128, HALF], dt.bfloat16, tag="tc1")
        so0 = persist.tile([128, HALF], dt.bfloat16, tag="so0")
        so1 = persist.tile([128, HALF], dt.bfloat16, tag="so1")

        xall = persist.tile([128, TS * HALF], dt.bfloat16, tag="xall")
        nc.vector.memset(xall, 0.0)
        for p_ in (1, 32, 65):
            nc.sync.dma_start(out=xall[p_:p_ + 1, :], in_=wONB[:])

        ms8 = [persist.tile([4, 8 * HALF], dt.bfloat16, tag=f"ms8{i}", name=f"ms8{i}")
               for i in range(2)]

        xt = persist.tile([TS, BN], dt.bfloat16, tag="xt")

        # stats + raw data per chunk (kept for the post-pass)
        full_c, mv_c, stdev_c, istd_c = [], [], [], []

        # ---- pre-pass: stats, normalize, transpose x ----------------------
        for c in range(NCHUNK):
            b_, n0 = c // 2, (c % 2) * 128
            raw = work.tile([T, 128], dt.float32, tag="raw")
            nc.sync.dma_start(out=raw[0:LIN, :], in_=hist[b_, :, n0:n0 + 128])
            nc.sync.dma_start(out=raw[LIN:T, :], in_=fut[b_, :, n0:n0 + 128])
            fpt = psl1.tile([128, T], dt.float32, tag="l1", name="fpt")
            nc.tensor.transpose(fpt, raw, IDF[0:T, 0:T])
            fc = persist.tile([128, T], dt.float32, tag=f"full{c}")
            nc.vector.tensor_copy(fc, fpt)
            st6 = work.tile([128, 6], dt.float32, tag="st6")
            mv = persist.tile([128, 2], dt.float32, tag=f"mv{c}")
            nc.vector.bn_stats(out=st6, in_=fc[:, 0:LIN])
            nc.vector.bn_aggr(out=mv, in_=st6)
            # stdev = sqrt(var+1e-5) with one Newton refinement (Sqrt table is
            # low precision); istd = 1/stdev via DVE reciprocal.
            veps = work.tile([128, 1], dt.float32, tag="veps")
            nc.vector.tensor_scalar(out=veps, in0=mv[:, 1:2], scalar1=1e-5,
                                    scalar2=None, op0=OP.add)
            y0 = work.tile([128, 1], dt.float32, tag="y0")
            nc.scalar.activation(y0, veps, AF.Sqrt)
            r0 = work.tile([128, 1], dt.float32, tag="r0")
            nc.vector.reciprocal(r0, y0)
            yy = work.tile([128, 1], dt.float32, tag="yy")
            nc.vector.tensor_tensor(out=yy, in0=y0, in1=y0, op=OP.mult)
            e_ = work.tile([128, 1], dt.float32, tag="e_")
            nc.vector.tensor_tensor(out=e_, in0=veps, in1=yy, op=OP.subtract)
            d_ = work.tile([128, 1], dt.float32, tag="d_")
            nc.vector.scalar_tensor_tensor(out=d_, in0=e_, scalar=0.5, in1=r0,
                                           op0=OP.mult, op1=OP.mult)
            sd = persist.tile([128, 1], dt.float32, tag=f"sd{c}")
            nc.vector.tensor_tensor(out=sd, in0=y0, in1=d_, op=OP.add)
            isd = persist.tile([128, 1], dt.float32, tag=f"isd{c}")
            nc.vector.reciprocal(isd, sd)
            full_c.append(fc); mv_c.append(mv); stdev_c.append(sd); istd_c.append(isd)

            # normalized x for steps 0..118, bf16
            xn = work.tile([128, TS], dt.bfloat16, tag="xn")
            nc.vector.tensor_scalar(out=xn, in0=fc[:, 0:TS], scalar1=mv[:, 0:1],
                                    scalar2=isd, op0=OP.subtract, op1=OP.mult)
            # transpose to [TS, 128] and write into xt with halves swapped
            pt = psl0.tile([TS, 128], dt.bfloat16, tag="l0", name="pt")
            nc.tensor.transpose(pt, xn, ID)
            xtcol = (1 - c // 4) * HALF + (c % 4) * 128
            nc.vector.tensor_copy(xt[:, xtcol:xtcol + 128], pt)

        # stage xt to DRAM, then load x rows into xall partitions 0 / 64
        nc.sync.dma_start(out=xs_d[:], in_=xt[:])
        nc.sync.dma_start(
            out=xall[0:1, :].rearrange("p (t b) -> p t b", b=HALF),
            in_=xs_d[None, :, 0:HALF])
        nc.sync.dma_start(
            out=xall[64:65, :].rearrange("p (t b) -> p t b", b=HALF),
            in_=xs_d[None, :, HALF:BN])

        # ---- main loop ----------------------------------------------------
        GS = [slice(X * 128, (X + 1) * 128) for X in range(4)]

        def lstm_matmuls(ps, Wh, hsrc, layer1, ts_):
            for X in range(4):
                psX = ps[:, X * HALF:(X + 1) * HALF]
                g = GS[X]
                nc.tensor.matmul(psX, lhsT=Wh[:, g], rhs=hsrc[:, :],
                                 start=True, stop=False)
                if layer1:
                    nc.tensor.matmul(psX, lhsT=WH1[:, g], rhs=ht1[:, :],
                                     start=False, stop=False)
                    nc.tensor.matmul(psX, lhsT=B1T[32:33, g],
                                     rhs=xall[32:33, ts_:ts_ + HALF],
                                     start=False, stop=True,
                                     tile_position=(32, 0))
                else:
                    nc.tensor.matmul(psX, lhsT=WX0[0:66, g],
                                     rhs=xall[0:66, ts_:ts_ + HALF],
                                     start=False, stop=True)

        def cell_update(tgb, ct, ub, vb, tcb, sob, htile):
            si = tgb[:, 0:HALF]
            sf = tgb[:, HALF:2 * HALF]
            tg_ = tgb[:, 2 * HALF:3 * HALF]
            to2 = tgb[:, 3 * HALF:4 * HALF]
            nc.vector.tensor_tensor(out=ub, in0=si, in1=tg_, op=OP.mult)
            nc.vector.tensor_tensor(out=vb, in0=sf, in1=ct, op=OP.mult)
            nc.vector.tensor_tensor(out=ct, in0=ub, in1=vb, op=OP.add)
            nc.scalar.activation(tcb, ct, AF.Tanh)
            nc.vector.tensor_scalar(out=sob, in0=to2, scalar1=0.5, scalar2=0.5,
                                    op0=OP.mult, op1=OP.add)
            nc.vector.tensor_tensor(out=htile, in0=sob, in1=tcb, op=OP.mult)

        for t in range(TS):
            ts_ = t * HALF
            l0ps = psl0.tile([128, 4 * HALF], dt.float32, tag="l0")
            lstm_matmuls(l0ps, WH0, ht0, layer1=False, ts_=ts_)
            nc.scalar.activation(tg0[:, 0:2 * HALF], l0ps[:, 0:2 * HALF],
                                 AF.Sigmoid)
            nc.scalar.activation(tg0[:, 2 * HALF:4 * HALF],
                                 l0ps[:, 2 * HALF:4 * HALF], AF.Tanh)
            cell_update(tg0, c0t, a0, b0, tc0, so0, ht0)

            l1ps = psl1.tile([128, 4 * HALF], dt.float32, tag="l1")
            lstm_matmuls(l1ps, WI1, ht0, layer1=True, ts_=ts_)
            nc.scalar.activation(tg1[:, 0:2 * HALF], l1ps[:, 0:2 * HALF],
                                 AF.Sigmoid)
            nc.scalar.activation(tg1[:, 2 * HALF:4 * HALF],
                                 l1ps[:, 2 * HALF:4 * HALF], AF.Tanh)
            cell_update(tg1, c1t, a1, b1t_, tc1, so1, ht1)

            hps = psl1.tile([4, HALF], dt.float32, tag="l1", name="hps")
            nc.tensor.matmul(hps, lhsT=HD[:, 0:4], rhs=ht1[:, :],
                             start=True, stop=True)
            ring = ms8[(t // 8) % 2]
            nc.vector.tensor_copy(ring[:, (t % 8) * HALF:(t % 8 + 1) * HALF], hps)
            if t % 8 == 7 or t == TS - 1:
                k0 = t - (t % 8)
                nw = t - k0 + 1
                nc.sync.dma_start(
                    out=musig_d[:, k0:t + 1, :],
                    in_=ring[:, 0:nw * HALF].rearrange("h (s b) -> h s b", b=HALF))

        # ---- post-pass ----------------------------------------------------
        for c in range(NCHUNK):
            b_, n0 = c // 2, (c % 2) * 128
            fc, mv, sd, isd = full_c[c], mv_c[c], stdev_c[c], istd_c[c]

            mu_tf = work.tile([128, 128], dt.bfloat16, tag="mu_tf")
            sg_tf = work.tile([128, 128], dt.bfloat16, tag="sg_tf")
            nc.sync.dma_start_transpose(out=mu_tf, in_=musig_d[0 + 2 * (c // 4), :, (c % 4) * 128:(c % 4 + 1) * 128])
            nc.sync.dma_start_transpose(out=sg_tf, in_=musig_d[1 + 2 * (c // 4), :, (c % 4) * 128:(c % 4 + 1) * 128])
            mu_t = mu_tf[:, 0:TS]
            sg_t = sg_tf[:, 0:TS]

            eps_c = work.tile([128, TS], dt.float32, tag="eps_c")
            nc.sync.dma_start(out=eps_c,
                              in_=epsin[b_, :, n0:n0 + 128].rearrange("t n -> n t"))
            mk = work.tile([128, TS], dt.float32, tag="mk")
            nc.sync.dma_start(out=mk[:, 0:LIN - 1],
                              in_=hmask[b_, 1:LIN, n0:n0 + 128].rearrange("t n -> n t"))
            nc.sync.dma_start(out=mk[:, LIN - 1:TS],
                              in_=fmask[b_, :, n0:n0 + 128].rearrange("t n -> n t"))

            # sigma = softplus(sg + sigma_b) + 1e-6  (stable exp/log form)
            ab_ = work.tile([128, TS], dt.float32, tag="ab_")
            nc.scalar.activation(ab_, sg_t, AF.Abs, bias=c_sigb)
            ex_ = work.tile([128, TS], dt.float32, tag="ex_")
            nc.scalar.activation(ex_, ab_, AF.Exp, scale=c_neg1)
            ln_ = work.tile([128, TS], dt.float32, tag="ln_")
            nc.scalar.activation(ln_, ex_, AF.Ln, bias=1.0)
            rl_ = work.tile([128, TS], dt.float32, tag="rl_")
            nc.vector.tensor_scalar(out=rl_, in0=sg_t, scalar1=sigma_b,
                                    scalar2=0.0, op0=OP.add, op1=OP.max)
            sig = work.tile([128, TS], dt.float32, tag="sig")
            nc.vector.scalar_tensor_tensor(out=sig, in0=ln_, scalar=1e-6, in1=rl_,
                                           op0=OP.add, op1=OP.add)

            # preds = ((mu+mu_b) + sigma*eps)*stdev + means, masked
            m1 = work.tile([128, TS], dt.float32, tag="m1")
            nc.vector.tensor_tensor(out=m1, in0=sig, in1=eps_c, op=OP.mult)
            m2 = work.tile([128, TS], dt.float32, tag="m2")
            nc.vector.scalar_tensor_tensor(out=m2, in0=mu_t, scalar=mu_b, in1=m1,
                                           op0=OP.add, op1=OP.add)
            m3 = work.tile([128, TS], dt.float32, tag="m3")
            nc.vector.tensor_scalar(out=m3, in0=m2, scalar1=sd, scalar2=mv[:, 0:1],
                                    op0=OP.mult, op1=OP.add)
            pr = work.tile([128, TS], dt.float32, tag="pr")
            nc.vector.tensor_tensor(out=pr, in0=m3, in1=mk, op=OP.mult)

            rr = work.tile([128, TS], dt.float32, tag="rr")
            nc.vector.tensor_tensor(out=rr, in0=fc[:, 1:T], in1=mk, op=OP.mult)

            u1 = work.tile([128, TS], dt.float32, tag="u1")
            nc.vector.tensor_scalar(out=u1, in0=mu_t, scalar1=mu_b, scalar2=None,
                                    op0=OP.add)
            u2 = work.tile([128, TS], dt.float32, tag="u2")
            nc.vector.tensor_scalar(out=u2, in0=u1, scalar1=sd, scalar2=mv[:, 0:1],
                                    op0=OP.mult, op1=OP.add)

            v1 = work.tile([128, TS], dt.float32, tag="v1")
            nc.vector.tensor_scalar(out=v1, in0=sig, scalar1=sd, scalar2=mv[:, 0:1],
                                    op0=OP.mult, op1=OP.add)

            # transpose each output [n,t] -> [t,n] on PE, then contiguous DMA
            for src_t, odram in ((pr, o_preds), (rr, o_reals), (u2, o_mus),
                                 (v1, o_sigs), (mk, o_mask)):
                tps = psl0.tile([TS, 128], dt.float32, tag="l0", name="tps")
                nc.tensor.transpose(tps, src_t, IDF)
                osb = work.tile([TS, 128], dt.float32, tag="osb", bufs=4)
                nc.vector.tensor_copy(osb, tps)
                nc.sync.dma_start(out=odram[b_, :, n0:n0 + 128], in_=osb)

    nc.finalize()
    return nc


def kernel(**inputs):
    import os
    from concourse.bass_utils import run_bass_kernel_spmd

    f32 = np.float32
    packs = _pack_weights(inputs)

    key = "nc"
    if key not in _cache:
        _cache[key] = _build(packs["mu_b"], packs["sigma_b"])
    nc = _cache[key]

    hist = np.ascontiguousarray(np.asarray(inputs["history_data"], f32)[..., 0])
    fut = np.ascontiguousarray(np.asarray(inputs["future_data"], f32)[..., 0])
    hm = np.ascontiguousarray(np.asarray(inputs["history_mask"], f32))
    fm = np.ascontiguousarray(np.asarray(inputs["future_mask"], f32))
    eps = np.ascontiguousarray(np.asarray(inputs["eps"], f32)[..., 0])

    in_maps = []
    for c in range(NCORES):
        b0, b1 = c * BL, (c + 1) * BL
        m = {
            "hist": hist[b0:b1], "fut": fut[b0:b1],
            "hmask": hm[b0:b1], "fmask": fm[b0:b1], "epsin": eps[b0:b1],
        }
        for k in ("WH0", "WX0", "WI1", "WH1", "B1T", "HD", "ID", "ONESBIG", "IDF"):
            m[k] = packs[k]
        in_maps.append(m)

    kres = run_bass_kernel_spmd(nc, in_maps, list(range(NCORES)),
                                trace=bool(os.environ.get("KERNEL_TRACE")))
    _cache["last"] = kres
    res = kres.results

    def gather(name):
        full = np.concatenate([res[c][name] for c in range(NCORES)], axis=0)
        return full.reshape(B, TS, N, 1).astype(f32)

    return (gather("preds"), gather("reals"), gather("musv"),
            gather("sigmasv"), gather("maskv"))


# revision 21
# speedup vs baseline: 3.0964x; 1.2328x over previous
"""DeepAR Trainium2 Bass kernel.

Strategy (hardcoded from spec nn_DeepAR_90374701843258):
  B=32, LIN=96, LOUT=24, N=256, E=32, H=64, T-1=119 steps, 8 cores.
  Data-parallel over B: 4 batch rows per core -> per-core batch BN=1024.
  Layout: "folded" [128, 512] tiles everywhere: partition p<64 = H-unit p of
  batch half 0 (cols 0:512 of the 1024 batch), p>=64 = H-unit p-64 of half 1.

  Algebra:
   - embedding + layer0 input proj collapse to rank-1: pre0 = x*w_eff + b_eff
     (w_eff = Wih0 @ embed_W), injected as extra contraction rows in the
     recurrent matmul (x and ones rows live in the xo tile).
   - sigmoid(z) = (tanh(z/2)+1)/2: i,f,o gate weights are pre-scaled by 0.5 so
     ALL four gates use one Tanh activation pass. h is stored as Hs=2h (the
     next-layer weights absorb the 0.5), cell state as s=2c.
     s' = 0.5*(tf+1)*s + (ti+1)*tg ; Hs = (to+1)*tanh(0.5*s')
     -> 4 fused scalar_tensor_tensor DVE ops per layer.
"""

import numpy as np

B, LIN, LOUT, N, E, H = 32, 96, 24, 256, 32, 64
T = LIN + LOUT
TS = T - 1            # 119
NCORES = 8
BL = B // NCORES      # 4
BN = BL * N           # 1024
HALF = 512
NCHUNK = BN // 128    # 8

_cache = {}


def _pack_weights(inp):
    """Host-side weight prep (tiny arrays). Returns dict of np arrays.

    Block-diagonal stationary layout: each gate's matmul processes BOTH folded
    batch halves in one K=128 matmul with lhsT = diag(W_X^T, W_X^T).
    Gates i,f,g natural (real sigmoid/tanh on ACT); o-gate pre-scaled by 0.5
    (sigmoid(o) = 0.5*tanh(o/2)+0.5 computed on DVE). h and c natural.
    """
    import ml_dtypes
    bf16 = ml_dtypes.bfloat16
    f32 = np.float32

    Wih0, Whh0 = inp["Wih0"].astype(f32), inp["Whh0"].astype(f32)
    Wih1, Whh1 = inp["Wih1"].astype(f32), inp["Whh1"].astype(f32)
    w_eff = (Wih0 @ inp["embed_W"].astype(f32))[:, 0]
    b_eff = Wih0 @ inp["embed_b"].astype(f32) + inp["bih0"] + inp["bhh0"]
    b1 = (inp["bih1"] + inp["bhh1"]).astype(f32)

    sc = np.ones(4 * H, f32)
    sc[3 * H:] = 0.5       # o-gate only

    def blockdiag(Wm):
        # Wm [4H, H]; returns [128, 4*128]
        out = np.zeros((128, 4 * 128), f32)
        for X in range(4):
            wt = (Wm[X * H:(X + 1) * H].T * sc[X * H:(X + 1) * H][None, :])
            out[0:64, X * 128:X * 128 + 64] = wt
            out[64:128, X * 128 + 64:(X + 1) * 128] = wt
        return out

    WH0 = blockdiag(Whh0)
    WI1 = blockdiag(Wih1)
    WH1 = blockdiag(Whh1)

    WX0 = np.zeros((128, 4 * 128), f32)
    for X in range(4):
        we = w_eff[X * H:(X + 1) * H] * sc[X * H:(X + 1) * H]
        be = b_eff[X * H:(X + 1) * H] * sc[X * H:(X + 1) * H]
        WX0[0, X * 128 + 64:(X + 1) * 128] = we   # x half1 -> out parts 64:128
        WX0[1, X * 128 + 64:(X + 1) * 128] = be
        WX0[64, X * 128:X * 128 + 64] = we        # x half0 -> out parts 0:64
        WX0[65, X * 128:X * 128 + 64] = be

    B1F = np.zeros((128, 4), f32)
    for X in range(4):
        bb = b1[X * H:(X + 1) * H] * sc[X * H:(X + 1) * H]
        B1F[0:64, X] = bb
        B1F[64:128, X] = bb

    HD = np.zeros((128, 4), f32)
    HD[0:64, 0] = inp["mu_W"].astype(f32)[0]
    HD[0:64, 1] = inp["sigma_W"].astype(f32)[0]
    HD[64:128, 2] = inp["mu_W"].astype(f32)[0]
    HD[64:128, 3] = inp["sigma_W"].astype(f32)[0]

    return {
        "WH0": WH0.astype(bf16), "WX0": WX0.astype(bf16),
        "WI1": WI1.astype(bf16), "WH1": WH1.astype(bf16),
        "B1F": B1F, "HD": HD.astype(bf16),
        "ID": np.eye(128, dtype=f32).astype(bf16),
        "ONESBIG": np.ones((1, TS * HALF), f32).astype(bf16),
        "IDF": np.eye(128, dtype=f32),
        "mu_b": float(inp["mu_b"][0]), "sigma_b": float(inp["sigma_b"][0]),
    }


def _build(mu_b, sigma_b):
    """Build the per-core bass program (SPMD: identical on all cores)."""
    from contextlib import ExitStack
    import concourse.bass as bass
    import concourse.mybir as mybir
    import concourse.tile as tile
    from concourse import bacc

    dt = mybir.dt
    AF = mybir.ActivationFunctionType
    OP = mybir.AluOpType

    nc = bacc.Bacc()

    # ---- I/O declarations -------------------------------------------------
    hist = nc.declare_dram_parameter("hist", [BL, LIN, N], dt.float32, isOutput=False)
    fut = nc.declare_dram_parameter("fut", [BL, LOUT, N], dt.float32, isOutput=False)
    hmask = nc.declare_dram_parameter("hmask", [BL, LIN, N], dt.float32, isOutput=False)
    fmask = nc.declare_dram_parameter("fmask", [BL, LOUT, N], dt.float32, isOutput=False)
    epsin = nc.declare_dram_parameter("epsin", [BL, TS, N], dt.float32, isOutput=False)
    wWH0 = nc.declare_dram_parameter("WH0", [128, 512], dt.bfloat16, isOutput=False)
    wWX0 = nc.declare_dram_parameter("WX0", [128, 512], dt.bfloat16, isOutput=False)
    wWI1 = nc.declare_dram_parameter("WI1", [128, 512], dt.bfloat16, isOutput=False)
    wWH1 = nc.declare_dram_parameter("WH1", [128, 512], dt.bfloat16, isOutput=False)
    wB1F = nc.declare_dram_parameter("B1F", [128, 4], dt.float32, isOutput=False)
    wHD = nc.declare_dram_parameter("HD", [128, 4], dt.bfloat16, isOutput=False)
    wID = nc.declare_dram_parameter("ID", [128, 128], dt.bfloat16, isOutput=False)
    wONB = nc.declare_dram_parameter("ONESBIG", [1, TS * HALF], dt.bfloat16, isOutput=False)
    wIDF = nc.declare_dram_parameter("IDF", [128, 128], dt.float32, isOutput=False)

    o_preds = nc.declare_dram_parameter("preds", [BL, TS, N], dt.float32, isOutput=True)
    o_reals = nc.declare_dram_parameter("reals", [BL, TS, N], dt.float32, isOutput=True)
    o_mus = nc.declare_dram_parameter("musv", [BL, TS, N], dt.float32, isOutput=True)
    o_sigs = nc.declare_dram_parameter("sigmasv", [BL, TS, N], dt.float32, isOutput=True)
    o_mask = nc.declare_dram_parameter("maskv", [BL, TS, N], dt.float32, isOutput=True)

    musig_d = nc.dram_tensor("musig", [4, 128, HALF], dt.bfloat16)
    xs_d = nc.dram_tensor("xsd", [TS, BN], dt.bfloat16)

    with ExitStack() as ctx:
        tc = ctx.enter_context(tile.TileContext(nc))
        persist = ctx.enter_context(tc.tile_pool(name="persist", bufs=1))
        work = ctx.enter_context(tc.tile_pool(name="work", bufs=3))
        psl0 = ctx.enter_context(tc.tile_pool(name="psl0", bufs=1, space="PSUM"))
        psl1 = ctx.enter_context(tc.tile_pool(name="psl1", bufs=1, space="PSUM"))

        # ---- constants / weights into SBUF -------------------------------
        WH0 = persist.tile([128, 512], dt.bfloat16, tag="WH0")
        WX0 = persist.tile([128, 512], dt.bfloat16, tag="WX0")
        WI1 = persist.tile([128, 512], dt.bfloat16, tag="WI1")
        WH1 = persist.tile([128, 512], dt.bfloat16, tag="WH1")
        B1F = persist.tile([128, 4], dt.float32, tag="B1F")
        HD = persist.tile([128, 4], dt.bfloat16, tag="HD")
        ID = persist.tile([128, 128], dt.bfloat16, tag="ID")
        IDF = persist.tile([128, 128], dt.float32, tag="IDF")
        for t_, d_ in [(WH0, wWH0), (WX0, wWX0), (WI1, wWI1), (WH1, wWH1),
                       (B1F, wB1F), (HD, wHD), (ID, wID), (IDF, wIDF)]:
            nc.sync.dma_start(out=t_[:], in_=d_[:])

        c_half = persist.tile([128, 1], dt.float32, tag="c_half")
        nc.vector.memset(c_half, 0.5)
        c_neg1 = persist.tile([128, 1], dt.float32, tag="c_neg1")
        nc.vector.memset(c_neg1, -1.0)
        c_sigb = persist.tile([128, 1], dt.float32, tag="c_sigb")
        nc.vector.memset(c_sigb, sigma_b)

        # ---- persistent state tiles ---------------------------------------
        ht0 = persist.tile([128, HALF], dt.bfloat16, tag="ht0")
        ht1 = persist.tile([128, HALF], dt.bfloat16, tag="ht1")
        c0t = persist.tile([128, HALF], dt.bfloat16, tag="c0t")
        c1t = persist.tile([128, HALF], dt.bfloat16, tag="c1t")
        for t_ in (ht0, ht1, c0t, c1t):
            nc.vector.memset(t_, 0.0)

        tg0 = persist.tile([128, 4 * HALF], dt.bfloat16, tag="tg0")
        tg1 = persist.tile([128, 4 * HALF], dt.bfloat16, tag="tg1")
        a0 = persist.tile([128, HALF], dt.bfloat16, tag="a0")
        b0 = persist.tile([128, HALF], dt.bfloat16, tag="b0")
        a1 = persist.tile([128, HALF], dt.bfloat16, tag="a1")
        b1t_ = persist.tile([128, HALF], dt.bfloat16, tag="b1t_")
        tc0 = persist.tile([128, HALF], dt.bfloat16, tag="tc0")
        tc1 = persist.tile([128, HALF], dt.bfloat16, tag="tc1")
        so0 = persist.tile([128, HALF], dt.bfloat16, tag="so0")
        so1 = persist.tile([128, HALF], dt.bfloat16, tag="so1")

        xall = persist.tile([128, TS * HALF], dt.bfloat16, tag="xall")
        nc.vector.memset(xall, 0.0)
        for p_ in (1, 32, 65):
            nc.sync.dma_start(out=xall[p_:p_ + 1, :], in_=wONB[:])

        ms8 = [persist.tile([4, 8 * HALF], dt.bfloat16, tag=f"ms8{i}", name=f"ms8{i}")
               for i in range(2)]

        xt = persist.tile([TS, BN], dt.bfloat16, tag="xt")

        # stats + raw data per chunk (kept for the post-pass)
        full_c, mv_c, stdev_c, istd_c = [], [], [], []

        # ---- pre-pass: stats, normalize, transpose x ----------------------
        for c in range(NCHUNK):
            b_, n0 = c // 2, (c % 2) * 128
            raw = work.tile([T, 128], dt.float32, tag="raw")
            nc.sync.dma_start(out=raw[0:LIN, :], in_=hist[b_, :, n0:n0 + 128])
            nc.sync.dma_start(out=raw[LIN:T, :], in_=fut[b_, :, n0:n0 + 128])
            fpt = psl1.tile([128, T], dt.float32, tag="l1", name="fpt")
            nc.tensor.transpose(fpt, raw, IDF[0:T, 0:T])
            fc = persist.tile([128, T], dt.float32, tag=f"full{c}")
            nc.vector.tensor_copy(fc, fpt)
            st6 = work.tile([128, 6], dt.float32, tag="st6")
            mv = persist.tile([128, 2], dt.float32, tag=f"mv{c}")
            nc.vector.bn_stats(out=st6, in_=fc[:, 0:LIN])
            nc.vector.bn_aggr(out=mv, in_=st6)
            # stdev = sqrt(var+1e-5) with one Newton refinement (Sqrt table is
            # low precision); istd = 1/stdev via DVE reciprocal.
            veps = work.tile([128, 1], dt.float32, tag="veps")
            nc.vector.tensor_scalar(out=veps, in0=mv[:, 1:2], scalar1=1e-5,
                                    scalar2=None, op0=OP.add)
            y0 = work.tile([128, 1], dt.float32, tag="y0")
            nc.scalar.activation(y0, veps, AF.Sqrt)
            r0 = work.tile([128, 1], dt.float32, tag="r0")
            nc.vector.reciprocal(r0, y0)
            yy = work.tile([128, 1], dt.float32, tag="yy")
            nc.vector.tensor_tensor(out=yy, in0=y0, in1=y0, op=OP.mult)
            e_ = work.tile([128, 1], dt.float32, tag="e_")
            nc.vector.tensor_tensor(out=e_, in0=veps, in1=yy, op=OP.subtract)
            d_ = work.tile([128, 1], dt.float32, tag="d_")
            nc.vector.scalar_tensor_tensor(out=d_, in0=e_, scalar=0.5, in1=r0,
                                           op0=OP.mult, op1=OP.mult)
            sd = persist.tile([128, 1], dt.float32, tag=f"sd{c}")
            nc.vector.tensor_tensor(out=sd, in0=y0, in1=d_, op=OP.add)
            isd = persist.tile([128, 1], dt.float32, tag=f"isd{c}")
            nc.vector.reciprocal(isd, sd)
            full_c.append(fc); mv_c.append(mv); stdev_c.append(sd); istd_c.append(isd)

            # normalized x for steps 0..118, bf16
            xn = work.tile([128, TS], dt.bfloat16, tag="xn")
            nc.vector.tensor_scalar(out=xn, in0=fc[:, 0:TS], scalar1=mv[:, 0:1],
                                    scalar2=isd, op0=OP.subtract, op1=OP.mult)
            # transpose to [TS, 128] and write into xt with halves swapped
            pt = psl0.tile([TS, 128], dt.bfloat16, tag="l0", name="pt")
            nc.tensor.transpose(pt, xn, ID)
            xtcol = (1 - c // 4) * HALF + (c % 4) * 128
            nc.vector.tensor_copy(xt[:, xtcol:xtcol + 128], pt)

        # stage xt to DRAM, then load x rows into xall partitions 0 / 64
        nc.sync.dma_start(out=xs_d[:], in_=xt[:])
        nc.sync.dma_start(
            out=xall[0:1, :].rearrange("p (t b) -> p t b", b=HALF),
            in_=xs_d[None, :, 0:HALF])
        nc.sync.dma_start(
            out=xall[64:65, :].rearrange("p (t b) -> p t b", b=HALF),
            in_=xs_d[None, :, HALF:BN])

        # ---- main loop ----------------------------------------------------
        GS = [slice(X * 128, (X + 1) * 128) for X in range(4)]

        def lstm_matmuls(ps, Wh, hsrc, layer1, ts_):
            for X in range(4):
                psX = ps[:, X * HALF:(X + 1) * HALF]
                g = GS[X]
                nc.tensor.matmul(psX, lhsT=Wh[:, g], rhs=hsrc[:, :],
                                 start=True, stop=False)
                if layer1:
                    nc.tensor.matmul(psX, lhsT=WH1[:, g], rhs=ht1[:, :],
                                     start=False, stop=True)
                else:
                    nc.tensor.matmul(psX, lhsT=WX0[0:66, g],
                                     rhs=xall[0:66, ts_:ts_ + HALF],
                                     start=False, stop=True)

        def cell_update(tgb, ct, ub, vb, tcb, sob, htile):
            si = tgb[:, 0:HALF]
            sf = tgb[:, HALF:2 * HALF]
            tg_ = tgb[:, 2 * HALF:3 * HALF]
            to2 = tgb[:, 3 * HALF:4 * HALF]
            nc.vector.tensor_tensor(out=ub, in0=si, in1=tg_, op=OP.mult)
            nc.vector.tensor_tensor(out=vb, in0=sf, in1=ct, op=OP.mult)
            nc.vector.tensor_tensor(out=ct, in0=ub, in1=vb, op=OP.add)
            nc.scalar.activation(tcb, ct, AF.Tanh)
            nc.vector.tensor_scalar(out=sob, in0=to2, scalar1=0.5, scalar2=0.5,
                                    op0=OP.mult, op1=OP.add)
            nc.vector.tensor_tensor(out=htile, in0=sob, in1=tcb, op=OP.mult)

        def heads_mm(t):
            hps = psl1.tile([4, HALF], dt.float32, tag="l1", name="hps")
            nc.tensor.matmul(hps, lhsT=HD[:, 0:4], rhs=ht1[:, :],
                             start=True, stop=True)
            ring = ms8[(t // 8) % 2]
            nc.vector.tensor_copy(ring[:, (t % 8) * HALF:(t % 8 + 1) * HALF], hps)
            if t % 8 == 7 or t == TS - 1:
                k0 = t - (t % 8)
                nw = t - k0 + 1
                nc.sync.dma_start(
                    out=musig_d[:, k0:t + 1, :],
                    in_=ring[:, 0:nw * HALF].rearrange("h (s b) -> h s b", b=HALF))

        for t in range(TS):
            ts_ = t * HALF
            l0ps = psl0.tile([128, 4 * HALF], dt.float32, tag="l0")
            lstm_matmuls(l0ps, WH0, ht0, layer1=False, ts_=ts_)
            if t > 0:
                heads_mm(t - 1)
            nc.scalar.activation(tg0[:, 0:2 * HALF], l0ps[:, 0:2 * HALF],
                                 AF.Sigmoid)
            nc.scalar.activation(tg0[:, 2 * HALF:4 * HALF],
                                 l0ps[:, 2 * HALF:4 * HALF], AF.Tanh)
            cell_update(tg0, c0t, a0, b0, tc0, so0, ht0)

            l1ps = psl1.tile([128, 4 * HALF], dt.float32, tag="l1")
            lstm_matmuls(l1ps, WI1, ht0, layer1=True, ts_=ts_)
            for X, fn in ((0, AF.Sigmoid), (1, AF.Sigmoid), (2, AF.Tanh),
                          (3, AF.Tanh)):
                nc.scalar.activation(tg1[:, X * HALF:(X + 1) * HALF],
                                     l1ps[:, X * HALF:(X + 1) * HALF], fn,
                                     bias=B1F[:, X:X + 1])
            cell_update(tg1, c1t, a1, b1t_, tc1, so1, ht1)
        heads_mm(TS - 1)

        # ---- post-pass ----------------------------------------------------
        for c in range(NCHUNK):
            b_, n0 = c // 2, (c % 2) * 128
            fc, mv, sd, isd = full_c[c], mv_c[c], stdev_c[c], istd_c[c]

            mu_tf = work.tile([128, 128], dt.bfloat16, tag="mu_tf")
            sg_tf = work.tile([128, 128], dt.bfloat16, tag="sg_tf")
            nc.sync.dma_start_transpose(out=mu_tf, in_=musig_d[0 + 2 * (c // 4), :, (c % 4) * 128:(c % 4 + 1) * 128])
            nc.sync.dma_start_transpose(out=sg_tf, in_=musig_d[1 + 2 * (c // 4), :, (c % 4) * 128:(c % 4 + 1) * 128])
            mu_t = mu_tf[:, 0:TS]
            sg_t = sg_tf[:, 0:TS]

            eps_c = work.tile([128, TS], dt.float32, tag="eps_c")
            nc.sync.dma_start(out=eps_c,
                              in_=epsin[b_, :, n0:n0 + 128].rearrange("t n -> n t"))
            mk = work.tile([128, TS], dt.float32, tag="mk")
            nc.sync.dma_start(out=mk[:, 0:LIN - 1],
                              in_=hmask[b_, 1:LIN, n0:n0 + 128].rearrange("t n -> n t"))
            nc.sync.dma_start(out=mk[:, LIN - 1:TS],
                              in_=fmask[b_, :, n0:n0 + 128].rearrange("t n -> n t"))

            # sigma = softplus(sg + sigma_b) + 1e-6  (stable exp/log form)
            ab_ = work.tile([128, TS], dt.float32, tag="ab_")
            nc.scalar.activation(ab_, sg_t, AF.Abs, bias=c_sigb)
            ex_ = work.tile([128, TS], dt.float32, tag="ex_")
            nc.scalar.activation(ex_, ab_, AF.Exp, scale=c_neg1)
            ln_ = work.tile([128, TS], dt.float32, tag="ln_")
            nc.scalar.activation(ln_, ex_, AF.Ln, bias=1.0)
            rl_ = work.tile([128, TS], dt.float32, tag="rl_")
            nc.vector.tensor_scalar(out=rl_, in0=sg_t, scalar1=sigma_b,
                                    scalar2=0.0, op0=OP.add, op1=OP.max)
            sig = work.tile([128, TS], dt.float32, tag="sig")
            nc.vector.scalar_tensor_tensor(out=sig, in0=ln_, scalar=1e-6, in1=rl_,
                                           op0=OP.add, op1=OP.add)

            # preds = ((mu+mu_b) + sigma*eps)*stdev + means, masked
            m1 = work.tile([128, TS], dt.float32, tag="m1")
            nc.vector.tensor_tensor(out=m1, in0=sig, in1=eps_c, op=OP.mult)
            m2 = work.tile([128, TS], dt.float32, tag="m2")
            nc.vector.scalar_tensor_tensor(out=m2, in0=mu_t, scalar=mu_b, in1=m1,
                                           op0=OP.add, op1=OP.add)
            m3 = work.tile([128, TS], dt.float32, tag="m3")
            nc.vector.tensor_scalar(out=m3, in0=m2, scalar1=sd, scalar2=mv[:, 0:1],
                                    op0=OP.mult, op1=OP.add)
            pr = work.tile([128, TS], dt.float32, tag="pr")
            nc.vector.tensor_tensor(out=pr, in0=m3, in1=mk, op=OP.mult)

            rr = work.tile([128, TS], dt.float32, tag="rr")
            nc.vector.tensor_tensor(out=rr, in0=fc[:, 1:T], in1=mk, op=OP.mult)

            u1 = work.tile([128, TS], dt.float32, tag="u1")
            nc.vector.tensor_scalar(out=u1, in0=mu_t, scalar1=mu_b, scalar2=None,
                                    op0=OP.add)
            u2 = work.tile([128, TS], dt.float32, tag="u2")
            nc.vector.tensor_scalar(out=u2, in0=u1, scalar1=sd, scalar2=mv[:, 0:1],
                                    op0=OP.mult, op1=OP.add)

            v1 = work.tile([128, TS], dt.float32, tag="v1")
            nc.vector.tensor_scalar(out=v1, in0=sig, scalar1=sd, scalar2=mv[:, 0:1],
                                    op0=OP.mult, op1=OP.add)

            # transpose each output [n,t] -> [t,n] on PE, then contiguous DMA
            for src_t, odram in ((pr, o_preds), (rr, o_reals), (u2, o_mus),
                                 (v1, o_sigs), (mk, o_mask)):
                tps = psl0.tile([TS, 128], dt.float32, tag="l0", name="tps")
                nc.tensor.transpose(tps, src_t, IDF)
                osb = work.tile([TS, 128], dt.float32, tag="osb", bufs=4)
                nc.vector.tensor_copy(osb, tps)
                nc.sync.dma_start(out=odram[b_, :, n0:n0 + 128], in_=osb)

    nc.finalize()
    return nc


def kernel(**inputs):
    import os
    from concourse.bass_utils import run_bass_kernel_spmd

    f32 = np.float32
    packs = _pack_weights(inputs)

    key = "nc"
    if key not in _cache:
        _cache[key] = _build(packs["mu_b"], packs["sigma_b"])
    nc = _cache[key]

    hist = np.ascontiguousarray(np.asarray(inputs["history_data"], f32)[..., 0])
    fut = np.ascontiguousarray(np.asarray(inputs["future_data"], f32)[..., 0])
    hm = np.ascontiguousarray(np.asarray(inputs["history_mask"], f32))
    fm = np.ascontiguousarray(np.asarray(inputs["future_mask"], f32))
    eps = np.ascontiguousarray(np.asarray(inputs["eps"], f32)[..., 0])

    in_maps = []
    for c in range(NCORES):
        b0, b1 = c * BL, (c + 1) * BL
        m = {
            "hist": hist[b0:b1], "fut": fut[b0:b1],
            "hmask": hm[b0:b1], "fmask": fm[b0:b1], "epsin": eps[b0:b1],
        }
        for k in ("WH0", "WX0", "WI1", "WH1", "B1F", "HD", "ID", "ONESBIG", "IDF"):
            m[k] = packs[k]
        in_maps.append(m)

    kres = run_bass_kernel_spmd(nc, in_maps, list(range(NCORES)),
                                trace=bool(os.environ.get("KERNEL_TRACE")))
    _cache["last"] = kres
    res = kres.results

    def gather(name):
        full = np.concatenate([res[c][name] for c in range(NCORES)], axis=0)
        return full.reshape(B, TS, N, 1).astype(f32)

    return (gather("preds"), gather("reals"), gather("musv"),
            gather("sigmasv"), gather("maskv"))


# revision 22
# speedup vs baseline: 3.2896x; 1.0624x over previous
"""DeepAR Trainium2 Bass kernel.

Strategy (hardcoded from spec nn_DeepAR_90374701843258):
  B=32, LIN=96, LOUT=24, N=256, E=32, H=64, T-1=119 steps, 8 cores.
  Data-parallel over B: 4 batch rows per core -> per-core batch BN=1024.
  Layout: "folded" [128, 512] tiles everywhere: partition p<64 = H-unit p of
  batch half 0 (cols 0:512 of the 1024 batch), p>=64 = H-unit p-64 of half 1.

  Algebra:
   - embedding + layer0 input proj collapse to rank-1: pre0 = x*w_eff + b_eff
     (w_eff = Wih0 @ embed_W), injected as extra contraction rows in the
     recurrent matmul (x and ones rows live in the xo tile).
   - sigmoid(z) = (tanh(z/2)+1)/2: i,f,o gate weights are pre-scaled by 0.5 so
     ALL four gates use one Tanh activation pass. h is stored as Hs=2h (the
     next-layer weights absorb the 0.5), cell state as s=2c.
     s' = 0.5*(tf+1)*s + (ti+1)*tg ; Hs = (to+1)*tanh(0.5*s')
     -> 4 fused scalar_tensor_tensor DVE ops per layer.
"""

import numpy as np

B, LIN, LOUT, N, E, H = 32, 96, 24, 256, 32, 64
T = LIN + LOUT
TS = T - 1            # 119
NCORES = 8
BL = B // NCORES      # 4
BN = BL * N           # 1024
HALF = 512
NCHUNK = BN // 128    # 8

_cache = {}


def _pack_weights(inp):
    """Host-side weight prep (tiny arrays). Returns dict of np arrays.

    Block-diagonal stationary layout: each gate's matmul processes BOTH folded
    batch halves in one K=128 matmul with lhsT = diag(W_X^T, W_X^T).
    Gates i,f,g natural (real sigmoid/tanh on ACT); o-gate pre-scaled by 0.5
    (sigmoid(o) = 0.5*tanh(o/2)+0.5 computed on DVE). h and c natural.
    """
    import ml_dtypes
    bf16 = ml_dtypes.bfloat16
    f32 = np.float32

    Wih0, Whh0 = inp["Wih0"].astype(f32), inp["Whh0"].astype(f32)
    Wih1, Whh1 = inp["Wih1"].astype(f32), inp["Whh1"].astype(f32)
    w_eff = (Wih0 @ inp["embed_W"].astype(f32))[:, 0]
    b_eff = Wih0 @ inp["embed_b"].astype(f32) + inp["bih0"] + inp["bhh0"]
    b1 = (inp["bih1"] + inp["bhh1"]).astype(f32)

    sc = np.ones(4 * H, f32)
    sc[3 * H:] = 0.5       # o-gate only

    def blockdiag(Wm):
        # Wm [4H, H]; returns [128, 4*128]
        out = np.zeros((128, 4 * 128), f32)
        for X in range(4):
            wt = (Wm[X * H:(X + 1) * H].T * sc[X * H:(X + 1) * H][None, :])
            out[0:64, X * 128:X * 128 + 64] = wt
            out[64:128, X * 128 + 64:(X + 1) * 128] = wt
        return out

    WH0 = blockdiag(Whh0)
    WI1 = blockdiag(Wih1)
    WH1 = blockdiag(Whh1)

    WX0 = np.zeros((128, 4 * 128), f32)
    for X in range(4):
        we = w_eff[X * H:(X + 1) * H] * sc[X * H:(X + 1) * H]
        be = b_eff[X * H:(X + 1) * H] * sc[X * H:(X + 1) * H]
        WX0[0, X * 128 + 64:(X + 1) * 128] = we   # x half1 -> out parts 64:128
        WX0[1, X * 128 + 64:(X + 1) * 128] = be
        WX0[64, X * 128:X * 128 + 64] = we        # x half0 -> out parts 0:64
        WX0[65, X * 128:X * 128 + 64] = be

    B1F = np.zeros((128, 4), f32)
    for X in range(4):
        bb = b1[X * H:(X + 1) * H] * sc[X * H:(X + 1) * H]
        B1F[0:64, X] = bb
        B1F[64:128, X] = bb

    HD = np.zeros((128, 4), f32)
    HD[0:64, 0] = inp["mu_W"].astype(f32)[0]
    HD[0:64, 1] = inp["sigma_W"].astype(f32)[0]
    HD[64:128, 2] = inp["mu_W"].astype(f32)[0]
    HD[64:128, 3] = inp["sigma_W"].astype(f32)[0]

    return {
        "WH0": WH0.astype(bf16), "WX0": WX0.astype(bf16),
        "WI1": WI1.astype(bf16), "WH1": WH1.astype(bf16),
        "B1F": B1F, "HD": HD.astype(bf16),
        "ID": np.eye(128, dtype=f32).astype(bf16),
        "ONESBIG": np.ones((1, TS * HALF), f32).astype(bf16),
        "IDF": np.eye(128, dtype=f32),
        "mu_b": float(inp["mu_b"][0]), "sigma_b": float(inp["sigma_b"][0]),
    }


def _build(mu_b, sigma_b):
    """Build the per-core bass program (SPMD: identical on all cores)."""
    from contextlib import ExitStack
    import concourse.bass as bass
    import concourse.mybir as mybir
    import concourse.tile as tile
    from concourse import bacc

    dt = mybir.dt
    AF = mybir.ActivationFunctionType
    OP = mybir.AluOpType

    nc = bacc.Bacc()

    # ---- I/O declarations -------------------------------------------------
    hist = nc.declare_dram_parameter("hist", [BL, LIN, N], dt.float32, isOutput=False)
    fut = nc.declare_dram_parameter("fut", [BL, LOUT, N], dt.float32, isOutput=False)
    hmask = nc.declare_dram_parameter("hmask", [BL, LIN, N], dt.float32, isOutput=False)
    fmask = nc.declare_dram_parameter("fmask", [BL, LOUT, N], dt.float32, isOutput=False)
    epsin = nc.declare_dram_parameter("epsin", [BL, TS, N], dt.float32, isOutput=False)
    wWH0 = nc.declare_dram_parameter("WH0", [128, 512], dt.bfloat16, isOutput=False)
    wWX0 = nc.declare_dram_parameter("WX0", [128, 512], dt.bfloat16, isOutput=False)
    wWI1 = nc.declare_dram_parameter("WI1", [128, 512], dt.bfloat16, isOutput=False)
    wWH1 = nc.declare_dram_parameter("WH1", [128, 512], dt.bfloat16, isOutput=False)
    wB1F = nc.declare_dram_parameter("B1F", [128, 4], dt.float32, isOutput=False)
    wHD = nc.declare_dram_parameter("HD", [128, 4], dt.bfloat16, isOutput=False)
    wID = nc.declare_dram_parameter("ID", [128, 128], dt.bfloat16, isOutput=False)
    wONB = nc.declare_dram_parameter("ONESBIG", [1, TS * HALF], dt.bfloat16, isOutput=False)
    wIDF = nc.declare_dram_parameter("IDF", [128, 128], dt.float32, isOutput=False)

    o_preds = nc.declare_dram_parameter("preds", [BL, TS, N], dt.float32, isOutput=True)
    o_reals = nc.declare_dram_parameter("reals", [BL, TS, N], dt.float32, isOutput=True)
    o_mus = nc.declare_dram_parameter("musv", [BL, TS, N], dt.float32, isOutput=True)
    o_sigs = nc.declare_dram_parameter("sigmasv", [BL, TS, N], dt.float32, isOutput=True)
    o_mask = nc.declare_dram_parameter("maskv", [BL, TS, N], dt.float32, isOutput=True)

    musig_d = nc.dram_tensor("musig", [4, 128, HALF], dt.bfloat16)
    xs_d = nc.dram_tensor("xsd", [TS, BN], dt.bfloat16)

    with ExitStack() as ctx:
        tc = ctx.enter_context(tile.TileContext(nc))
        persist = ctx.enter_context(tc.tile_pool(name="persist", bufs=1))
        work = ctx.enter_context(tc.tile_pool(name="work", bufs=3))
        psl0 = ctx.enter_context(tc.tile_pool(name="psl0", bufs=1, space="PSUM"))
        psl1 = ctx.enter_context(tc.tile_pool(name="psl1", bufs=1, space="PSUM"))

        # ---- constants / weights into SBUF -------------------------------
        WH0 = persist.tile([128, 512], dt.bfloat16, tag="WH0")
        WX0 = persist.tile([128, 512], dt.bfloat16, tag="WX0")
        WI1 = persist.tile([128, 512], dt.bfloat16, tag="WI1")
        WH1 = persist.tile([128, 512], dt.bfloat16, tag="WH1")
        B1F = persist.tile([128, 4], dt.float32, tag="B1F")
        HD = persist.tile([128, 4], dt.bfloat16, tag="HD")
        ID = persist.tile([128, 128], dt.bfloat16, tag="ID")
        IDF = persist.tile([128, 128], dt.float32, tag="IDF")
        for t_, d_ in [(WH0, wWH0), (WX0, wWX0), (WI1, wWI1), (WH1, wWH1),
                       (B1F, wB1F), (HD, wHD), (ID, wID), (IDF, wIDF)]:
            nc.sync.dma_start(out=t_[:], in_=d_[:])

        c_half = persist.tile([128, 1], dt.float32, tag="c_half")
        nc.vector.memset(c_half, 0.5)
        c_neg1 = persist.tile([128, 1], dt.float32, tag="c_neg1")
        nc.vector.memset(c_neg1, -1.0)
        c_sigb = persist.tile([128, 1], dt.float32, tag="c_sigb")
        nc.vector.memset(c_sigb, sigma_b)

        # ---- persistent state tiles ---------------------------------------
        ht0 = persist.tile([128, HALF], dt.bfloat16, tag="ht0")
        ht1 = persist.tile([128, HALF], dt.bfloat16, tag="ht1")
        c0t = persist.tile([128, HALF], dt.bfloat16, tag="c0t")
        c1t = persist.tile([128, HALF], dt.bfloat16, tag="c1t")
        for t_ in (ht0, ht1, c0t, c1t):
            nc.vector.memset(t_, 0.0)

        tg0 = persist.tile([128, 4 * HALF], dt.bfloat16, tag="tg0")
        tg1 = persist.tile([128, 4 * HALF], dt.bfloat16, tag="tg1")
        a0 = persist.tile([128, HALF], dt.bfloat16, tag="a0")
        b0 = persist.tile([128, HALF], dt.bfloat16, tag="b0")
        a1 = persist.tile([128, HALF], dt.bfloat16, tag="a1")
        b1t_ = persist.tile([128, HALF], dt.bfloat16, tag="b1t_")
        tc0 = persist.tile([128, HALF], dt.bfloat16, tag="tc0")
        tc1 = persist.tile([128, HALF], dt.bfloat16, tag="tc1")
        so0 = persist.tile([128, HALF], dt.bfloat16, tag="so0")
        so1 = persist.tile([128, HALF], dt.bfloat16, tag="so1")

        xall = persist.tile([128, TS * HALF], dt.bfloat16, tag="xall")
        nc.vector.memset(xall, 0.0)
        for p_ in (1, 32, 65):
            nc.sync.dma_start(out=xall[p_:p_ + 1, :], in_=wONB[:])

        ms8 = [persist.tile([4, 8 * HALF], dt.bfloat16, tag=f"ms8{i}", name=f"ms8{i}")
               for i in range(2)]

        xt = persist.tile([TS, BN], dt.bfloat16, tag="xt")

        # stats + raw data per chunk (kept for the post-pass)
        full_c, mv_c, stdev_c, istd_c = [], [], [], []

        # ---- pre-pass: stats, normalize, transpose x ----------------------
        for c in range(NCHUNK):
            b_, n0 = c // 2, (c % 2) * 128
            raw = work.tile([T, 128], dt.float32, tag="raw")
            nc.sync.dma_start(out=raw[0:LIN, :], in_=hist[b_, :, n0:n0 + 128])
            nc.sync.dma_start(out=raw[LIN:T, :], in_=fut[b_, :, n0:n0 + 128])
            fpt = psl1.tile([128, T], dt.float32, tag="l1", name="fpt")
            nc.tensor.transpose(fpt, raw, IDF[0:T, 0:T])
            fc = persist.tile([128, T], dt.float32, tag=f"full{c}")
            nc.vector.tensor_copy(fc, fpt)
            st6 = work.tile([128, 6], dt.float32, tag="st6")
            mv = persist.tile([128, 2], dt.float32, tag=f"mv{c}")
            nc.vector.bn_stats(out=st6, in_=fc[:, 0:LIN])
            nc.vector.bn_aggr(out=mv, in_=st6)
            # stdev = sqrt(var+1e-5) with one Newton refinement (Sqrt table is
            # low precision); istd = 1/stdev via DVE reciprocal.
            veps = work.tile([128, 1], dt.float32, tag="veps")
            nc.vector.tensor_scalar(out=veps, in0=mv[:, 1:2], scalar1=1e-5,
                                    scalar2=None, op0=OP.add)
            y0 = work.tile([128, 1], dt.float32, tag="y0")
            nc.scalar.activation(y0, veps, AF.Sqrt)
            r0 = work.tile([128, 1], dt.float32, tag="r0")
            nc.vector.reciprocal(r0, y0)
            yy = work.tile([128, 1], dt.float32, tag="yy")
            nc.vector.tensor_tensor(out=yy, in0=y0, in1=y0, op=OP.mult)
            e_ = work.tile([128, 1], dt.float32, tag="e_")
            nc.vector.tensor_tensor(out=e_, in0=veps, in1=yy, op=OP.subtract)
            d_ = work.tile([128, 1], dt.float32, tag="d_")
            nc.vector.scalar_tensor_tensor(out=d_, in0=e_, scalar=0.5, in1=r0,
                                           op0=OP.mult, op1=OP.mult)
            sd = persist.tile([128, 1], dt.float32, tag=f"sd{c}")
            nc.vector.tensor_tensor(out=sd, in0=y0, in1=d_, op=OP.add)
            isd = persist.tile([128, 1], dt.float32, tag=f"isd{c}")
            nc.vector.reciprocal(isd, sd)
            full_c.append(fc); mv_c.append(mv); stdev_c.append(sd); istd_c.append(isd)

            # normalized x for steps 0..118, bf16
            xn = work.tile([128, TS], dt.bfloat16, tag="xn")
            nc.vector.tensor_scalar(out=xn, in0=fc[:, 0:TS], scalar1=mv[:, 0:1],
                                    scalar2=isd, op0=OP.subtract, op1=OP.mult)
            # transpose to [TS, 128] and write into xt with halves swapped
            pt = psl0.tile([TS, 128], dt.bfloat16, tag="l0", name="pt")
            nc.tensor.transpose(pt, xn, ID)
            xtcol = (1 - c // 4) * HALF + (c % 4) * 128
            nc.vector.tensor_copy(xt[:, xtcol:xtcol + 128], pt)

        # stage xt to DRAM, then load x rows into xall partitions 0 / 64
        nc.sync.dma_start(out=xs_d[:], in_=xt[:])
        nc.sync.dma_start(
            out=xall[0:1, :].rearrange("p (t b) -> p t b", b=HALF),
            in_=xs_d[None, :, 0:HALF])
        nc.sync.dma_start(
            out=xall[64:65, :].rearrange("p (t b) -> p t b", b=HALF),
            in_=xs_d[None, :, HALF:BN])

        # ---- main loop ----------------------------------------------------
        GS = [slice(X * 128, (X + 1) * 128) for X in range(4)]

        def lstm_matmuls(ps, Wh, hsrc, layer1, ts_):
            for X in range(4):
                psX = ps[:, X * HALF:(X + 1) * HALF]
                g = GS[X]
                nc.tensor.matmul(psX, lhsT=Wh[:, g], rhs=hsrc[:, :],
                                 start=True, stop=False)
                if layer1:
                    nc.tensor.matmul(psX, lhsT=WH1[:, g], rhs=ht1[:, :],
                                     start=False, stop=True)
                else:
                    nc.tensor.matmul(psX, lhsT=WX0[0:66, g],
                                     rhs=xall[0:66, ts_:ts_ + HALF],
                                     start=False, stop=True)

        def cell_update(tgb, ct, ub, vb, tcb, sob, htile):
            si = tgb[:, 0:HALF]
            sf = tgb[:, HALF:2 * HALF]
            tg_ = tgb[:, 2 * HALF:3 * HALF]
            to2 = tgb[:, 3 * HALF:4 * HALF]
            nc.vector.tensor_tensor(out=ub, in0=si, in1=tg_, op=OP.mult)
            nc.vector.tensor_tensor(out=vb, in0=sf, in1=ct, op=OP.mult)
            nc.vector.tensor_tensor(out=ct, in0=ub, in1=vb, op=OP.add)
            nc.scalar.activation(tcb, ct, AF.Tanh)
            nc.vector.tensor_scalar(out=sob, in0=to2, scalar1=0.5, scalar2=0.5,
                                    op0=OP.mult, op1=OP.add)
            nc.vector.tensor_tensor(out=htile, in0=sob, in1=tcb, op=OP.mult)

        def heads_mm(t):
            hps = psl0.tile([4, HALF], dt.float32, tag="l0", name="hps")
            nc.tensor.matmul(hps, lhsT=HD[:, 0:4], rhs=ht1[:, :],
                             start=True, stop=True)
            ring = ms8[(t // 8) % 2]
            nc.vector.tensor_copy(ring[:, (t % 8) * HALF:(t % 8 + 1) * HALF], hps)
            if t % 8 == 7 or t == TS - 1:
                k0 = t - (t % 8)
                nw = t - k0 + 1
                nc.sync.dma_start(
                    out=musig_d[:, k0:t + 1, :],
                    in_=ring[:, 0:nw * HALF].rearrange("h (s b) -> h s b", b=HALF))

        for t in range(TS):
            ts_ = t * HALF
            l0ps = psl0.tile([128, 4 * HALF], dt.float32, tag="l0")
            lstm_matmuls(l0ps, WH0, ht0, layer1=False, ts_=ts_)
            if t > 0:
                heads_mm(t - 1)
            for X, fn in ((0, AF.Sigmoid), (2, AF.Tanh), (1, AF.Sigmoid),
                          (3, AF.Tanh)):
                nc.scalar.activation(tg0[:, X * HALF:(X + 1) * HALF],
                                     l0ps[:, X * HALF:(X + 1) * HALF], fn)
            cell_update(tg0, c0t, a0, b0, tc0, so0, ht0)

            l1ps = psl1.tile([128, 4 * HALF], dt.float32, tag="l1")
            lstm_matmuls(l1ps, WI1, ht0, layer1=True, ts_=ts_)
            for X, fn in ((0, AF.Sigmoid), (2, AF.Tanh), (1, AF.Sigmoid),
                          (3, AF.Tanh)):
                nc.scalar.activation(tg1[:, X * HALF:(X + 1) * HALF],
                                     l1ps[:, X * HALF:(X + 1) * HALF], fn,
                                     bias=B1F[:, X:X + 1])
            cell_update(tg1, c1t, a1, b1t_, tc1, so1, ht1)
        heads_mm(TS - 1)

        # ---- post-pass ----------------------------------------------------
        for c in range(NCHUNK):
            b_, n0 = c // 2, (c % 2) * 128
            fc, mv, sd, isd = full_c[c], mv_c[c], stdev_c[c], istd_c[c]

            mu_tf = work.tile([128, 128], dt.bfloat16, tag="mu_tf")
            sg_tf = work.tile([128, 128], dt.bfloat16, tag="sg_tf")
            nc.sync.dma_start_transpose(out=mu_tf, in_=musig_d[0 + 2 * (c // 4), :, (c % 4) * 128:(c % 4 + 1) * 128])
            nc.sync.dma_start_transpose(out=sg_tf, in_=musig_d[1 + 2 * (c // 4), :, (c % 4) * 128:(c % 4 + 1) * 128])
            mu_t = mu_tf[:, 0:TS]
            sg_t = sg_tf[:, 0:TS]

            eps_c = work.tile([128, TS], dt.float32, tag="eps_c")
            nc.sync.dma_start(out=eps_c,
                              in_=epsin[b_, :, n0:n0 + 128].rearrange("t n -> n t"))
            mk = work.tile([128, TS], dt.float32, tag="mk")
            nc.sync.dma_start(out=mk[:, 0:LIN - 1],
                              in_=hmask[b_, 1:LIN, n0:n0 + 128].rearrange("t n -> n t"))
            nc.sync.dma_start(out=mk[:, LIN - 1:TS],
                              in_=fmask[b_, :, n0:n0 + 128].rearrange("t n -> n t"))

            # sigma = softplus(sg + sigma_b) + 1e-6  (stable exp/log form)
            ab_ = work.tile([128, TS], dt.float32, tag="ab_")
            nc.scalar.activation(ab_, sg_t, AF.Abs, bias=c_sigb)
            ex_ = work.tile([128, TS], dt.float32, tag="ex_")
            nc.scalar.activation(ex_, ab_, AF.Exp, scale=c_neg1)
            ln_ = work.tile([128, TS], dt.float32, tag="ln_")
            nc.scalar.activation(ln_, ex_, AF.Ln, bias=1.0)
            rl_ = work.tile([128, TS], dt.float32, tag="rl_")
            nc.vector.tensor_scalar(out=rl_, in0=sg_t, scalar1=sigma_b,
                                    scalar2=0.0, op0=OP.add, op1=OP.max)
            sig = work.tile([128, TS], dt.float32, tag="sig")
            nc.vector.scalar_tensor_tensor(out=sig, in0=ln_, scalar=1e-6, in1=rl_,
                                           op0=OP.add, op1=OP.add)

            # preds = ((mu+mu_b) + sigma*eps)*stdev + means, masked
            m1 = work.tile([128, TS], dt.float32, tag="m1")
            nc.vector.tensor_tensor(out=m1, in0=sig, in1=eps_c, op=OP.mult)
            m2 = work.tile([128, TS], dt.float32, tag="m2")
            nc.vector.scalar_tensor_tensor(out=m2, in0=mu_t, scalar=mu_b, in1=m1,
                                           op0=OP.add, op1=OP.add)
            m3 = work.tile([128, TS], dt.float32, tag="m3")
            nc.vector.tensor_scalar(out=m3, in0=m2, scalar1=sd, scalar2=mv[:, 0:1],
                                    op0=OP.mult, op1=OP.add)
            pr = work.tile([128, TS], dt.float32, tag="pr")
            nc.vector.tensor_tensor(out=pr, in0=m3, in1=mk, op=OP.mult)

            rr = work.tile([128, TS], dt.float32, tag="rr")
            nc.vector.tensor_tensor(out=rr, in0=fc[:, 1:T], in1=mk, op=OP.mult)

            u1 = work.tile([128, TS], dt.float32, tag="u1")
            nc.vector.tensor_scalar(out=u1, in0=mu_t, scalar1=mu_b, scalar2=None,
                                    op0=OP.add)
            u2 = work.tile([128, TS], dt.float32, tag="u2")
            nc.vector.tensor_scalar(out=u2, in0=u1, scalar1=sd, scalar2=mv[:, 0:1],
                                    op0=OP.mult, op1=OP.add)

            v1 = work.tile([128, TS], dt.float32, tag="v1")
            nc.vector.tensor_scalar(out=v1, in0=sig, scalar1=sd, scalar2=mv[:, 0:1],
                                    op0=OP.mult, op1=OP.add)

            # transpose each output [n,t] -> [t,n] on PE, then contiguous DMA
            for src_t, odram in ((pr, o_preds), (rr, o_reals), (u2, o_mus),
                                 (v1, o_sigs), (mk, o_mask)):
                tps = psl0.tile([TS, 128], dt.float32, tag="l0", name="tps")
                nc.tensor.transpose(tps, src_t, IDF)
                osb = work.tile([TS, 128], dt.float32, tag="osb", bufs=4)
                nc.vector.tensor_copy(osb, tps)
                nc.sync.dma_start(out=odram[b_, :, n0:n0 + 128], in_=osb)

    nc.finalize()
    return nc


def kernel(**inputs):
    import os
    from concourse.bass_utils import run_bass_kernel_spmd

    f32 = np.float32
    packs = _pack_weights(inputs)

    key = "nc"
    if key not in _cache:
        _cache[key] = _build(packs["mu_b"], packs["sigma_b"])
    nc = _cache[key]

    hist = np.ascontiguousarray(np.asarray(inputs["history_data"], f32)[..., 0])
    fut = np.ascontiguousarray(np.asarray(inputs["future_data"], f32)[..., 0])
    hm = np.ascontiguousarray(np.asarray(inputs["history_mask"], f32))
    fm = np.ascontiguousarray(np.asarray(inputs["future_mask"], f32))
    eps = np.ascontiguousarray(np.asarray(inputs["eps"], f32)[..., 0])

    in_maps = []
    for c in range(NCORES):
        b0, b1 = c * BL, (c + 1) * BL
        m = {
            "hist": hist[b0:b1], "fut": fut[b0:b1],
            "hmask": hm[b0:b1], "fmask": fm[b0:b1], "epsin": eps[b0:b1],
        }
        for k in ("WH0", "WX0", "WI1", "WH1", "B1F", "HD", "ID", "ONESBIG", "IDF"):
            m[k] = packs[k]
        in_maps.append(m)

    kres = run_bass_kernel_spmd(nc, in_maps, list(range(NCORES)),
                                trace=bool(os.environ.get("KERNEL_TRACE")))
    _cache["last"] = kres
    res = kres.results

    def gather(name):
        full = np.concatenate([res[c][name] for c in range(NCORES)], axis=0)
        return full.reshape(B, TS, N, 1).astype(f32)

    return (gather("preds"), gather("reals"), gather("musv"),
            gather("sigmasv"), gather("maskv"))
